# revision 75
# baseline (speedup 1.0000x reference)
"""Trainium2 Bass kernel for BEiT-3 multiway multihead attention, v3.

Strategy
--------
8-way data parallelism over batch (one batch element per NeuronCore).

PE-row-minimal design (the cost model charges matmuls by moving-stream
length only; stationary loads are free):

  qT/kT : W-stationary projections, feature-major [e, t] outputs.
  v     : x^T-stationary projections, token-major [s, e] outputs with an
          all-ones column per head (-> softmax denominators ride the P@V).
  scores: per head, kT-slice stationary [64, 128s], qT moving -> [s, t]
          fp32 PSUM; exp on Act (PSUM->SBUF bf16); multiplicative
          exp(mask) factor on DVE.
  P@V   : probs-slice STATIONARY [128s, 128t], v moving [128s, 65]
          -> token-major [t, 65] PSUM accumulated over s-chunks:
          65 rows/matmul instead of 512.
  norm  : per-token-chunk LN stats (head-sum tree + elementwise square +
          accumulate, engine-steered: GpSimd at the phase boundary, DVE in
          the tail); rstd = exp(-0.5 ln(var+eps)) so every activation stays
          in the single preloaded natural_log_exp_and_others table set (no
          mid-kernel act-table reloads); DMA xbar transpose of the applied
          chunk to feature-major.
  out   : feature-major out-projection - stationary Wg slices (gamma
          folded), moving centered-attn^T; quarter-granular accumulation
          groups gate on individual chunk transposes. Output is [E, T]
          bf16 (host transposes and casts up).

Group schedule is HALF-MAJOR: (pair, half) groups run (0,0)..(7,0),
(0,1)..(7,1) so token chunks 0-3 complete at the phase midpoint and their
LN/transpose/out-projection overlap the half-1 compute; out-projection of
half 0 becomes PE filler for the Act-bound half-1 phase. q/k/v projections
fill phase A (weights and per-pair wv slices prefetched one slot ahead;
need-ordered startup DMA). Scores run three groups ahead of P@V (probs
quadruple-buffered; half-0 q tiles rotate through 3 buffers).
"""

from contextlib import ExitStack

import numpy as np
import ml_dtypes

import concourse.bass as bass
import concourse.mybir as mybir
from concourse import bacc, tile
from concourse.bass import ts
from concourse.bass_utils import run_bass_kernel_spmd

AF = mybir.ActivationFunctionType
ALU = mybir.AluOpType

B = 8
E = 1024
T = 1024
H = 16
HD = 64
P = 128
NCH = E // P          # feature chunks (= head pairs)
NTC = T // P          # token chunks
EPS = 1e-5
BF16 = mybir.dt.bfloat16
F32 = mybir.dt.float32
F32R = mybir.dt.float32r
NPBF16 = ml_dtypes.bfloat16


def _segs(lo, hi, split):
    """Token segments [lo, hi) split by modality boundary. -> [(s0, s1, m)]"""
    out = []
    if lo < min(hi, split):
        out.append((lo, min(hi, split), 0))
    if max(lo, split) < hi:
        out.append((max(lo, split), hi, 1))
    return out


def build_module(split: int, v_bias: bool, qk_bias: bool = True, o_bias: bool = True):
    assert 0 <= split <= T and split % 32 == 0, split
    nc = bacc.Bacc("TRN2", target_bir_lowering=False, debug=False)

    xqT = nc.declare_dram_parameter("xqT", [P, NCH * T], BF16, isOutput=False)
    xkT = nc.declare_dram_parameter("xkT", [P, NCH * T], BF16, isOutput=False)
    xvT = nc.declare_dram_parameter("xvT", [P, NCH * T], BF16, isOutput=False)
    wq = nc.declare_dram_parameter("wq", [2, NCH, P, NCH * P], BF16, isOutput=False)
    wk = nc.declare_dram_parameter("wk", [2, NCH, P, NCH * P], BF16, isOutput=False)
    wg = nc.declare_dram_parameter("wg", [2, 2, P, NCH * 512], BF16, isOutput=False)
    wv = nc.declare_dram_parameter("wv", [2, 2, P, NCH * 512], BF16, isOutput=False)
    em = nc.declare_dram_parameter("em", [P, NCH * T], BF16, isOutput=False)
    bq = nc.declare_dram_parameter("bq", [2, E], F32, isOutput=False)
    bk = nc.declare_dram_parameter("bk", [2, E], F32, isOutput=False)
    bv = nc.declare_dram_parameter("bv", [2, E], F32R, isOutput=False)
    c2 = nc.declare_dram_parameter("c2", [2, E], F32, isOutput=False)
    ones_d = nc.declare_dram_parameter("ones_d", [1, P], F32R, isOutput=False)
    outT = nc.declare_dram_parameter("outT", [E, T], BF16, isOutput=True)

    used_m = sorted(set(m for _, _, m in _segs(0, T, split)))

    with tile.TileContext(nc) as tc, ExitStack() as ctx:
        # preload the ln+exp activation-function set once: Exp, Ln, Square,
        # Identity and Copy all live in natural_log_exp_and_others (id 6),
        # so the exp stream never swaps tables mid-kernel
        nc.scalar.add_instruction(
            mybir.InstLoadActFuncSet(
                name=nc.get_next_instruction_name(),
                ins=[], outs=[], act_func_set_id=6,
            )
        )
        const = ctx.enter_context(tc.tile_pool(name="const", bufs=1))
        eps_t = const.tile([P, 1], F32)
        nc.vector.memset(eps_t[:], EPS)
        bq_sb = bk_sb = c2_sb = None
        if qk_bias:
            bq_sb = const.tile([P, 2 * NCH], F32)
            bk_sb = const.tile([P, 2 * NCH], F32)
            for m in (0, 1):
                cs = slice(m * NCH, (m + 1) * NCH)
                nc.sync.dma_start(bq_sb[:, cs], bq[m].rearrange("(c p) -> p c", p=P))
                nc.sync.dma_start(bk_sb[:, cs], bk[m].rearrange("(c p) -> p c", p=P))
        if o_bias:
            c2_sb = const.tile([P, 2 * NCH], F32)
            for m in (0, 1):
                cs = slice(m * NCH, (m + 1) * NCH)
                nc.sync.dma_start(c2_sb[:, cs], c2[m].rearrange("(c p) -> p c", p=P))
        ones_row = None
        bv_row_sb = None
        if v_bias:
            ones_row = const.tile([1, P], F32R)
            nc.sync.dma_start(ones_row[:], ones_d[:])
            bv_row_sb = const.tile([1, 2 * E], F32R)
            for m in (0, 1):
                nc.sync.dma_start(bv_row_sb[0:1, m * E:(m + 1) * E], bv[m][None, :])

        # ---------------- long-lived SBUF pools --------------------------
        # attn chunks rotate through 4 tags: chunk 4+j reuses chunk j's
        # buffer once chunk j's LN-apply (its last reader) is done.
        attn_pool = ctx.enter_context(tc.tile_pool(name="attn", bufs=1))
        attn_sc_box = {}

        def attn_sc(jj):
            if jj not in attn_sc_box:
                attn_sc_box[jj] = attn_pool.tile(
                    [P, E], BF16, tag=f"asc{jj % 4}", name=f"asc{jj}")
            return attn_sc_box[jj]

        attn_T3_box = {}

        stats_pool = ctx.enter_context(tc.tile_pool(name="stats", bufs=1))
        sums = [stats_pool.tile([P, H], F32, tag=f"sums{j}", name=f"sums{j}")
                for j in range(NTC)]
        scr_pool = ctx.enter_context(tc.tile_pool(name="scr", bufs=2))
        apl_pool = ctx.enter_context(tc.tile_pool(name="apl", bufs=2))
        em_pool = ctx.enter_context(tc.tile_pool(name="em", bufs=2))

        # PSUM pools
        proj_ps = ctx.enter_context(tc.tile_pool(name="proj_ps", bufs=2, space="PSUM"))
        sc_pool = ctx.enter_context(tc.tile_pool(name="sc_ps", bufs=2, space="PSUM"))
        at_pool = ctx.enter_context(tc.tile_pool(name="at_ps", bufs=2, space="PSUM"))

        main = ExitStack()
        with main:
            qk_sb = main.enter_context(tc.tile_pool(name="qk_sb", bufs=1))
            # half-0 q tiles die right after their scores group: rotate 3
            qk0_pool = main.enter_context(tc.tile_pool(name="qk0", bufs=3))
            vem_pool = main.enter_context(tc.tile_pool(name="vem", bufs=1))
            # probs pipeline: 4 generations deep (scores three groups ahead)
            pr_poolA = main.enter_context(tc.tile_pool(name="probsA", bufs=4))
            pr_poolB = main.enter_context(tc.tile_pool(name="probsB", bufs=4))
            wqk_pool = main.enter_context(tc.tile_pool(name="wqk", bufs=2))
            x_stack = ExitStack()
            x_pool = x_stack.enter_context(tc.tile_pool(name="xpool", bufs=1))

            wqk = {}

            def fetch_wqk(eo):
                # both modalities in one DMA (3D AP) to halve HWDGE holds
                for name, w_dram in (("q", wq), ("k", wk)):
                    if len(used_m) == 2:
                        wt = wqk_pool.tile([P, 2, NCH * P], BF16,
                                           tag=f"w{name}", name=f"w{name}e{eo}")
                        nc.sync.dma_start(
                            wt[:], w_dram[:, eo].rearrange("m p c -> p m c"))
                        for m in (0, 1):
                            wqk[(name, m, eo)] = wt[:, m]
                    else:
                        m = used_m[0]
                        wt = wqk_pool.tile([P, NCH * P], BF16,
                                           tag=f"w{name}", name=f"w{name}e{eo}")
                        nc.sync.dma_start(wt[:], w_dram[m, eo])
                        wqk[(name, m, eo)] = wt

            xq_tile = x_pool.tile([P, NCH * T], BF16, tag="xq", name="xq")
            xk_tile = x_pool.tile([P, NCH * T], BF16, tag="xk", name="xk")

            def dma_x_half(xt, xd, h0):
                for c in range(0, NCH, 2):
                    nc.sync.dma_start(xt[:, c * T:(c + 2) * T].rearrange(
                        "p (c2 t) -> p c2 t", t=T)[:, :, h0:h0 + 512],
                        xd[:, c * T:(c + 2) * T].rearrange(
                        "p (c2 t) -> p c2 t", t=T)[:, :, h0:h0 + 512])

            # em is staged one token-half at a time into half-size tiles
            # (double-buffered so half 1 can stream in early).
            em_box = {}

            def dma_em_half(h0):
                emt = em_pool.tile([P, NCH * 512], BF16, tag="em",
                                   name=f"em{h0}")
                em_box[1 if h0 else 0] = [
                    emt[:, c * 512:(c + 1) * 512] for c in range(NCH)]
                for c in range(0, NCH, 2):
                    nc.sync.dma_start(
                        emt[:, c * 512:(c + 2) * 512].rearrange(
                            "p (c2 t) -> p c2 t", t=512),
                        em[:, c * T:(c + 2) * T].rearrange(
                            "p (c2 t) -> p c2 t", t=T)[:, :, h0:h0 + 512])

            # startup DMAs, need-ordered and finely interleaved so the first
            # projection matmuls start as early as possible: wq(0) -> first
            # xq chunks -> wk(0) -> xk chunks -> mask half 0
            def fetch_w_one(name, w_dram, eo):
                # two half-DMAs: the projection's c-loop starts after the
                # first 4 feature chunks land
                if len(used_m) == 2:
                    wt = wqk_pool.tile([P, 2, NCH * P], BF16,
                                       tag=f"w{name}", name=f"w{name}e{eo}")
                    for h in (0, 1):
                        cs = slice(h * 512, (h + 1) * 512)
                        nc.sync.dma_start(
                            wt[:, :, cs],
                            w_dram[:, eo].rearrange("m p c -> p m c")[:, :, cs])
                    for m in (0, 1):
                        wqk[(name, m, eo)] = wt[:, m]
                else:
                    m = used_m[0]
                    wt = wqk_pool.tile([P, NCH * P], BF16,
                                       tag=f"w{name}", name=f"w{name}e{eo}")
                    for h in (0, 1):
                        cs = slice(h * 512, (h + 1) * 512)
                        nc.sync.dma_start(wt[:, cs], w_dram[m, eo][:, cs])
                    wqk[(name, m, eo)] = wt

            fetch_w_one("q", wq, 0)
            dma_x_half(xq_tile, xqT, 0)
            fetch_w_one("k", wk, 0)
            dma_x_half(xk_tile, xkT, 0)
            dma_x_half(xk_tile, xkT, 512)
            dma_em_half(0)
            fetch_wqk(1)
            dma_x_half(xq_tile, xqT, 512)

            xq_t = [xq_tile[:, c * T:(c + 1) * T] for c in range(NCH)]
            xk_t = [xk_tile[:, c * T:(c + 1) * T] for c in range(NCH)]

            # v output tiles (token-major, 66 cols per head, ones at col 64)
            v_t = []
            for tc_ in range(NTC):
                vt = vem_pool.tile([P, H * 65], BF16, tag=f"v{tc_}", name=f"v{tc_}")
                nc.vector.memset(
                    vt[:].rearrange("p (g w) -> p g w", w=65)[:, :, 64:65], 1.0
                )
                v_t.append(vt)

            qT_t, kT_t = {}, {}
            stats_tiles = {}

            def st(nm, jj):
                key = (nm, jj)
                if key not in stats_tiles:
                    stats_tiles[key] = stats_pool.tile(
                        [P, 1], F32, tag=f"{nm}{jj}", name=f"{nm}{jj}")
                return stats_tiles[key]

            def emit_qk_half(eo, half, name):
                x_t = xq_t if name == "q" else xk_t
                b_sb = bq_sb if name == "q" else bk_sb
                lo = half * 512
                if name == "q":
                    # q halves live in separate 512-col tiles; half 0
                    # rotates through 3 buffers (dead after its scores)
                    if half == 0:
                        qtile = qk0_pool.tile([P, 512], BF16, tag="qT0",
                                              name=f"qT0e{eo}")
                        qT_t[(eo, 0)] = qtile
                    else:
                        qtile = qk_sb.tile([P, 512], BF16, tag=f"qT1{eo}",
                                           name=f"qT1e{eo}")
                        qT_t[(eo, 1)] = qtile
                    toff = -lo  # tile-local columns
                else:
                    if eo not in kT_t:
                        kT_t[eo] = qk_sb.tile([P, T], BF16, tag=f"kT{eo}",
                                              name=f"kT{eo}")
                    qtile = kT_t[eo]
                    toff = 0
                ps = proj_ps.tile([P, 512], F32, tag="pp", name="pp")
                for s0, s1, m in _segs(lo, lo + 512, split):
                    wt = wqk[(name, m, eo)]
                    for c in range(NCH):
                        nc.tensor.matmul(
                            ps[:, s0 - lo:s1 - lo],
                            wt[:, ts(c, P)],
                            x_t[c][:, s0:s1],
                            start=(c == 0),
                            stop=(c == NCH - 1),
                        )
                # PSUM evacuation must stay off GpSimd (no PSUM access)
                if qk_bias:
                    for s0, s1, m in _segs(lo, lo + 512, split):
                        nc.vector.tensor_scalar_add(
                            qtile[:, s0 + toff:s1 + toff],
                            ps[:, s0 - lo:s1 - lo],
                            b_sb[:, m * NCH + eo:m * NCH + eo + 1],
                        )
                else:
                    nc.vector.tensor_copy(
                        qtile[:, lo + toff:lo + toff + 512], ps[:])

            # xv/wv in their own stack frame, closed after the v projection
            xvwv = ExitStack()
            xv_pool = xvwv.enter_context(tc.tile_pool(name="xv_p", bufs=1))
            xv_tile = xv_pool.tile([P, NCH * T], BF16, tag="xv", name="xv")
            wv_sb = {}

            def dma_xv():
                for c in range(0, NCH, 2):
                    nc.sync.dma_start(xv_tile[:, c * T:(c + 2) * T],
                                      xvT[:, c * T:(c + 2) * T])

            xv_t = [xv_tile[:, c * T:(c + 1) * T] for c in range(NCH)]

            def fetch_wv_pair(pair):
                # only the 128 e_out columns this head pair needs, strided
                # out of the eoh-block layout
                eoh, q4 = pair // 4, pair % 4
                for m in used_m:
                    wt = xv_pool.tile([P, NCH * P], BF16, tag=f"wv{m}",
                                      name=f"wv{m}p{pair}")
                    nc.sync.dma_start(
                        wt[:].rearrange("p (c w) -> p c w", w=P),
                        wv[m, eoh].rearrange("p (c w) -> p c w", w=512)[
                            :, :, q4 * P:(q4 + 1) * P])
                    wv_sb[(m, pair)] = wt

            def emit_v_pair(pair):
                """V projection for one head pair (128 e_out cols) over all
                token groups; 4 token groups share one PSUM tile so the
                evacuation amortizes and never gates the matmul stream."""
                eoh, q4 = pair // 4, pair % 4
                for tq in (0, 1):
                    ps = proj_ps.tile([P, 512], F32, tag="pp", name="pp")
                    for gi in range(4):
                        tc_ = 4 * tq + gi
                        lo = tc_ * P
                        gs = slice(gi * P, (gi + 1) * P)
                        for s0, s1, m in _segs(lo, lo + P, split):
                            m0, m1 = s0 - lo, s1 - lo
                            tp = (0, m0) if m0 else None
                            wvt = wv_sb[(m, pair)]
                            for c in range(NCH):
                                nc.tensor.matmul(
                                    ps[m0:m1, gs],
                                    xv_t[c][:, s0:s1],
                                    wvt[:, c * P:(c + 1) * P],
                                    start=(c == 0),
                                    stop=(c == NCH - 1) and not v_bias,
                                    tile_position=tp,
                                )
                            if v_bias:
                                nc.tensor.matmul(
                                    ps[m0:m1, gs],
                                    ones_row[0:1, 0:m1 - m0],
                                    bv_row_sb[
                                        0:1,
                                        m * E + eoh * 512 + q4 * P:
                                        m * E + eoh * 512 + (q4 + 1) * P,
                                    ].bitcast(F32R),
                                    start=False,
                                    stop=True,
                                    tile_position=tp,
                                )
                    for gi in range(4):
                        tc_ = 4 * tq + gi
                        dst = v_t[tc_][:].rearrange("p (g w) -> p g w", w=65)[
                            :, 2 * pair:2 * pair + 2, 0:64
                        ]
                        src_ = ps[:, gi * P:(gi + 1) * P].rearrange(
                            "p (g w) -> p g w", w=64)
                        nc.vector.tensor_copy(dst, src_)

            wg_sb = {}

            def fetch_wg():
                wg_pool = main.enter_context(tc.tile_pool(name="wg_sb", bufs=1))
                attn_T = wg_pool.tile([P, NCH * T], BF16, tag="attnT",
                                      name="attnT")
                attn_T3_box[0] = attn_T[:].rearrange("p (c t) -> p c t", t=T)
                for eoh in (0, 1):
                    for m in used_m:
                        wt = wg_pool.tile([P, NCH * 512], BF16,
                                          tag=f"wg{m}{eoh}", name=f"wg{m}{eoh}")
                        nc.sync.dma_start(wt[:], wg[m, eoh])
                        wg_sb[(m, eoh)] = wt

            def scores_c(pair, half, c, mask_on_pool=False):
                """Scores+exp+mask for one s-chunk of group (pair, half)."""
                lo = half * 512
                sc = sc_pool.tile([P, 1024], F32, tag="sc", name="sc")
                qsrc = qT_t[(pair, half)]
                nc.tensor.matmul(
                    sc[:, 0:512],
                    kT_t[pair][0:HD, ts(c, P)],
                    qsrc[0:HD, 0:512],
                )
                nc.tensor.matmul(
                    sc[:, 512:1024],
                    kT_t[pair][HD:P, ts(c, P)],
                    qsrc[HD:P, 0:512],
                )
                pr_pool = pr_poolA if c < 4 else pr_poolB
                pr = pr_pool.tile([P, 1024], BF16, tag=f"pr{c}", name=f"pr{c}")
                nc.scalar.activation(pr[:], sc[:], AF.Exp)
                emc = em_box[half][c]
                eng = nc.gpsimd if mask_on_pool else nc.vector
                eng.tensor_mul(pr[:, 0:512], pr[:, 0:512], emc)
                eng.tensor_mul(pr[:, 512:1024], pr[:, 512:1024], emc)
                return pr

            def scores_group(pair, half):
                return [scores_c(pair, half, c) for c in range(NCH)]

            def chunk_stats(jj):
                """LN stats for token chunk jj: head sums -> mean; Act
                square (+fused accumulator for the tail chunks); per-chunk
                rstd and centering bias. For chunks 4-7 the small ops run
                on GpSimd/Act so the DVE stays on the P@V evac stream (the
                epilogue's pacing chain)."""
                tail = jj >= 4
                veng = nc.vector if tail else nc.gpsimd
                ap = apl_pool.tile([P, E], BF16, tag="apl", name=f"apl{jj}")
                stats_tiles[("apl", jj)] = ap
                # mean: one fused reduce over the 16 per-head sums (the
                # bf16 out scratch is the apply tile, overwritten next).
                # accum_out is DVE-only (invalid opcode on Pool).
                nc.vector.tensor_scalar(
                    ap[:, 0:H], sums[jj][:], 1.0, 0.0, ALU.mult, ALU.add,
                    accum_out=st("mus", jj)[:],
                )
                # square on Pool (boundary: Act+DVE saturated) / DVE (tail:
                # Act must not serialize the chunk chains); accumulate on DVE
                sq_eng = nc.vector if tail else nc.gpsimd
                sq_eng.tensor_tensor(ap[:], attn_sc(jj)[:], attn_sc(jj)[:],
                                     ALU.mult)
                nc.vector.tensor_scalar(
                    ap[:], ap[:], 1.0, 0.0, ALU.mult, ALU.add,
                    accum_out=st("sq", jj)[:],
                )
                mu2 = st("mu2", jj)
                veng.tensor_scalar(
                    mu2[:], st("mus", jj)[:], st("mus", jj)[:, 0:1],
                    1.0 / (E * E), ALU.mult, ALU.mult
                )
                var = st("var", jj)
                veng.tensor_scalar(
                    var[:], st("sq", jj)[:], 1.0 / E, mu2[:, 0:1],
                    ALU.mult, ALU.subtract
                )
                # rstd = exp(-0.5 ln(var+eps)): eps rides the Ln bias and
                # everything stays in the ln+exp act table set
                lnv = st("lnv", jj)
                nc.scalar.activation(lnv[:], var[:], AF.Ln,
                                     bias=eps_t[:, 0:1])
                rst = st("rst", jj)
                nc.scalar.activation(rst[:], lnv[:], AF.Exp, scale=-0.5)
                veng.tensor_scalar(
                    st("bsh", jj)[:], st("mus", jj)[:], rst[:, 0:1],
                    -1.0 / E, ALU.mult, ALU.mult
                )

            def apply_chunk(jj):
                """LN-apply for chunk jj. Chunks 0-3 run on GpSimd (Act and
                DVE are saturated at the phase boundary); chunks 4-7 run on
                DVE in 4x mode (idle in the tail, and Act must stay off the
                transpose-gating chain)."""
                ap = stats_tiles[("apl", jj)]
                eng = nc.vector if jj >= 4 else nc.gpsimd
                eng.tensor_scalar(
                    ap[:], attn_sc(jj)[:], st("rst", jj)[:, 0:1],
                    st("bsh", jj)[:, 0:1], ALU.mult, ALU.add,
                )

            def transpose_chunk(jj):
                nc.sync.dma_start_transpose(
                    attn_T3_box[0][:, :, jj * P:(jj + 1) * P],
                    stats_tiles[("apl", jj)][:],
                )

            def pv_j(pair, half, j, prs, with_stats=False):
                """One token-chunk of P@V for group (pair, half) with fused
                divide-normalize evacuation; on the last pair the LN stats +
                apply chain for chunk jj runs inline."""
                hA, hB = 2 * pair, 2 * pair + 1
                jj = half * 4 + j
                # both heads in one PSUM tile: one tag, double-buffered, so
                # the next pv_j's matmuls never wait on this one's evacs
                at = at_pool.tile([P, 130], F32, tag="at", name="at")
                for i, (h, toff) in enumerate(((hA, 0), (hB, 512))):
                    for c in range(NCH):
                        nc.tensor.matmul(
                            at[:, 65 * i:65 * i + 65],
                            prs[c][:, toff + j * P:toff + (j + 1) * P],
                            v_t[c][:, 65 * h:65 * h + 65],
                            start=(c == 0),
                            stop=(c == NCH - 1),
                        )
                den = stats_pool.tile([P, 2], F32, tag=f"den{j % 2}",
                                      name=f"den{j % 2}")
                for i in (0, 1):
                    nc.vector.reciprocal_approx_fast(
                        out=den[:, i:i + 1], in_=at[:, 65 * i + 64:65 * i + 65]
                    )
                for i, h in enumerate((hA, hB)):
                    nc.vector.tensor_scalar(
                        attn_sc(jj)[:, h * HD:(h + 1) * HD],
                        at[:, 65 * i:65 * i + HD],
                        den[:, i:i + 1],
                        0.0,
                        ALU.mult,
                        ALU.add,
                        accum_out=sums[jj][:, h:h + 1],
                    )
                if with_stats:
                    chunk_stats(jj)
                    apply_chunk(jj)
                    transpose_chunk(jj)

            def outproj_eo(half, eo, split_out=False):
                """Feature-major out-projection for one e_out chunk;
                quarter-granular accumulation groups gated on individual
                transposes. split_out pipelines the evacuation + store in
                halves (tail-drain)."""
                lo = half * 512
                ps = proj_ps.tile([P, 512], F32, tag="pp", name="pp")
                osb = scr_pool.tile([P, 512], BF16, tag="osb", name="osb")
                for q in range(4):
                    qlo = lo + q * P
                    for s0, s1, m in _segs(qlo, qlo + P, split):
                        wt = wg_sb[(m, eo // 4)]
                        for c in range(NCH):
                            nc.tensor.matmul(
                                ps[:, s0 - lo:s1 - lo],
                                wt[:].rearrange("p (c2 w) -> p c2 w", w=512)[
                                    :, c, (eo % 4) * P:(eo % 4 + 1) * P],
                                attn_T3_box[0][:, c, s0:s1],
                                start=(c == 0),
                                stop=(c == NCH - 1),
                            )
                    if split_out and q == 1:
                        nc.scalar.copy(osb[:, 0:256], ps[:, 0:256])
                        nc.sync.dma_start(
                            outT[ts(eo, P), lo:lo + 256], osb[:, 0:256])
                if o_bias:
                    for s0, s1, m in _segs(lo, lo + 512, split):
                        nc.scalar.activation(
                            osb[:, s0 - lo:s1 - lo], ps[:, s0 - lo:s1 - lo],
                            AF.Identity,
                            bias=c2_sb[:, m * NCH + eo:m * NCH + eo + 1],
                        )
                    nc.sync.dma_start(outT[ts(eo, P), lo:lo + 512], osb[:])
                elif split_out:
                    nc.scalar.copy(osb[:, 256:512], ps[:, 256:512])
                    nc.sync.dma_start(
                        outT[ts(eo, P), lo + 256:lo + 512], osb[:, 256:512])
                else:
                    if half == 0:
                        nc.vector.tensor_copy(osb[:], ps[:])
                    else:
                        # epilogue: Act idles once the exp stream has drained
                        nc.scalar.copy(osb[:], ps[:])
                    nc.sync.dma_start(outT[ts(eo, P), lo:lo + 512], osb[:])

            def pv_interleaved(pair, half, prs, nxt, fillers=(),
                               with_stats=False, mask_on_pool=False):
                """PV j-blocks for (pair, half) interleaved with the group
                two slots ahead's scores and PE filler work."""
                nxt_prs = []
                fillers = list(fillers)
                for j in range(4):
                    pv_j(pair, half, j, prs, with_stats=with_stats)
                    if fillers:
                        fillers.pop(0)()
                    if nxt is not None:
                        np_, nh = nxt
                        nxt_prs.append(scores_c(np_, nh, 2 * j, mask_on_pool))
                        nxt_prs.append(
                            scores_c(np_, nh, 2 * j + 1, mask_on_pool))
                for f in fillers:
                    f()
                return nxt_prs

            # ------------------------- group schedule ---------------------
            # prologue: full projections for pairs 0-2, scores for the first
            # three groups (the pipeline runs three groups ahead so the Act
            # exp stream never starves), v projection for pairs 0-1.
            prss = {}
            emit_qk_half(0, 0, "q")
            emit_qk_half(0, 0, "k")
            emit_qk_half(0, 1, "k")
            prss[(0, 0)] = [scores_c(0, 0, c) for c in range(4)]
            emit_qk_half(0, 1, "q")
            emit_qk_half(1, 0, "q")
            emit_qk_half(1, 0, "k")
            prss[(0, 0)] += [scores_c(0, 0, c) for c in range(4, 8)]
            emit_qk_half(1, 1, "k")
            emit_qk_half(1, 1, "q")
            dma_xv()
            fetch_wv_pair(0)
            fetch_wv_pair(1)
            prss[(1, 0)] = scores_group(1, 0)
            fetch_wqk(2)
            emit_qk_half(2, 0, "q")
            emit_qk_half(2, 0, "k")
            emit_v_pair(0)
            emit_qk_half(2, 1, "k")
            emit_qk_half(2, 1, "q")
            fetch_wqk(3)
            fetch_wv_pair(2)
            prss[(2, 0)] = scores_group(2, 0)
            emit_v_pair(1)
            dma_em_half(512)

            def qhs(eo):
                def f():
                    if eo + 1 < NCH:
                        fetch_wqk(eo + 1)
                    for half, name in ((0, "q"), (0, "k"), (1, "k"), (1, "q")):
                        emit_qk_half(eo, half, name)
                return f

            def vp(pair):
                def f():
                    emit_v_pair(pair)
                    if pair + 1 < NCH:
                        fetch_wv_pair(pair + 1)
                return f

            def op(half, eo):
                return lambda: outproj_eo(half, eo)

            def slot(p, h, np_, nh, fillers=(), with_stats=False,
                     mask_on_pool=False):
                nxt = (np_, nh) if np_ is not None else None
                nxt_prs = pv_interleaved(
                    p, h, prss.pop((p, h)), nxt, fillers,
                    with_stats=with_stats, mask_on_pool=mask_on_pool)
                if nxt is not None:
                    prss[nxt] = nxt_prs

            # ---- phase A: half-0 groups; q/k/v projections as filler ----
            # qhs(p) (as first filler) projects all four halves of pair p
            # right before that pair's scores and prefetches the next pair's
            # weights; vp(p) runs two slots ahead of its P@V group.
            slot(0, 0, 3, 0, [qhs(3), vp(2)])
            slot(1, 0, 4, 0, [qhs(4), vp(3)])
            slot(2, 0, 5, 0, [qhs(5), vp(4)])
            slot(3, 0, 6, 0, [qhs(6), vp(5)])
            slot(4, 0, 7, 0, [qhs(7), vp(6)])
            slot(5, 0, 0, 1, [vp(7)])
            xvwv.close()
            x_stack.close()
            fetch_wg()
            slot(6, 0, 1, 1)
            slot(7, 0, 2, 1, with_stats=True)

            # ---- phase B: half-1 groups; half-0 out-projection and the
            # deferred q-h1 projections fill the Act-bound slots.
            slot(0, 1, 3, 1, [op(0, 0)])
            slot(1, 1, 4, 1, [op(0, 1)])
            slot(2, 1, 5, 1, [op(0, 2)])
            slot(3, 1, 6, 1, [op(0, 3)])
            slot(4, 1, 7, 1, [op(0, 4)])
            slot(5, 1, None, None, [op(0, 5)])
            slot(6, 1, None, None, [op(0, 6)])
            outproj_eo(0, 7)

            # ---- epilogue: last group with inline stats+apply+transpose;
            # the half-1 out-projections' quarter accumulation groups gate
            # on the individual transposes, so emitting them after the last
            # transpose still overlaps the chunk 4-6 chains.
            prs71 = prss.pop((7, 1))
            pv_j(7, 1, 0, prs71, with_stats=True)
            pv_j(7, 1, 1, prs71, with_stats=True)
            pv_j(7, 1, 2, prs71, with_stats=True)
            pv_j(7, 1, 3, prs71, with_stats=True)
            for eo in range(NCH):
                outproj_eo(1, eo, split_out=(eo >= NCH - 2))

    nc.compile()
    return nc


def _pack_pmajor(arr2d):
    # [NCH*P, T] -> [P, NCH*T]: row p holds chunk-major concatenation
    return np.ascontiguousarray(
        arr2d.reshape(NCH, P, T).transpose(1, 0, 2).reshape(P, NCH * T)
    )


def _pack_wv_style(Wt, Wi):
    # [2, eoh, p, c*512+j] with arr[c*128+p, eoh*512+j], arr = W.T
    out = np.empty((2, 2, P, NCH * 512), NPBF16)
    for m, W in enumerate((Wt, Wi)):
        arr = (W.T).astype(NPBF16)  # [e_in, e_out]
        out[m] = (
            arr.reshape(NCH, P, 2, 512)
            .transpose(2, 1, 0, 3)
            .reshape(2, P, NCH * 512)
        )
    return np.ascontiguousarray(out)


def _host_prep(inputs):
    scaling = HD ** -0.5
    f32 = np.float32

    def a(name):
        return np.asarray(inputs[name], f32)

    def prep_blocks(Wt, Wi, scale=1.0):
        # [2, eo, p, c*128+j] with arr[c*128+p, eo*128+j]
        out = np.empty((2, NCH, P, NCH * P), NPBF16)
        for m, W in enumerate((Wt, Wi)):
            arr = ((W * scale).T).astype(NPBF16)  # [e_in, e_out]
            out[m] = (
                arr.reshape(NCH, P, NCH, P)
                .transpose(2, 1, 0, 3)
                .reshape(NCH, P, NCH * P)
            )
        return np.ascontiguousarray(out)

    Wo_t, Wo_i = a("Wo_t"), a("Wo_i")
    g_t, g_i = a("ln_g_t"), a("ln_g_i")
    b_t, b_i = a("ln_b_t"), a("ln_b_i")
    Wg_t = Wo_t * g_t[None, :]
    Wg_i = Wo_i * g_i[None, :]

    wq_np = prep_blocks(a("Wq_t"), a("Wq_i"), scaling)
    wk_np = prep_blocks(a("Wk_t"), a("Wk_i"))
    wg_np = _pack_wv_style(Wg_t, Wg_i)
    wv_np = _pack_wv_style(a("Wv_t"), a("Wv_i"))

    em_np = _pack_pmajor(
        np.exp(np.asarray(inputs["attention_mask"], np.float64)).T.astype(NPBF16)
    )

    bq_np = np.stack([a("bq_t"), a("bq_i")]) * f32(scaling)
    bk_np = np.stack([a("bk_t"), a("bk_i")])
    bv_np = np.stack([a("bv_t"), a("bv_i")])
    c2_np = np.stack(
        [
            Wo_t.astype(np.float64) @ b_t.astype(np.float64) + a("bo_t"),
            Wo_i.astype(np.float64) @ b_i.astype(np.float64) + a("bo_i"),
        ]
    ).astype(f32)
    ones_np = np.ones((1, P), np.float32)

    shared = dict(
        wq=wq_np, wk=wk_np, wg=wg_np, wv=wv_np, em=em_np,
        ones_d=ones_np,
        bq=np.ascontiguousarray(bq_np), bk=np.ascontiguousarray(bk_np),
        bv=np.ascontiguousarray(bv_np), c2=np.ascontiguousarray(c2_np),
    )
    flags = (
        bool(np.any(bv_np)),
        bool(np.any(bq_np) or np.any(bk_np)),
        bool(np.any(c2_np)),
    )
    return shared, flags


_CACHE = {}


def build_cached(split, flags):
    key = (split, flags)
    if key not in _CACHE:
        _CACHE[key] = build_module(split, *flags)
    return _CACHE[key]


def kernel(**inputs):
    q = np.asarray(inputs["query"], np.float32)
    k = np.asarray(inputs["key"], np.float32)
    v = np.asarray(inputs["value"], np.float32)
    assert q.shape == (B, T, E), q.shape
    split = int(np.asarray(inputs["split_position"]))

    shared, flags = _host_prep(inputs)
    nc = build_cached(split, flags)

    in_maps = []
    for b in range(B):
        m = dict(shared)
        m["xqT"] = _pack_pmajor(q[b].T.astype(NPBF16))
        m["xkT"] = _pack_pmajor(k[b].T.astype(NPBF16))
        m["xvT"] = _pack_pmajor(v[b].T.astype(NPBF16))
        in_maps.append(m)

    res = run_bass_kernel_spmd(nc, in_maps, list(range(B)))
    out = np.stack(
        [np.ascontiguousarray(res.results[b]["outT"].T) for b in range(B)]
    )
    return out.astype(np.float32)


# revision 78
# speedup vs baseline: 1.0150x; 1.0150x over previous
"""Trainium2 Bass kernel for BEiT-3 multiway multihead attention, v3.

Strategy
--------
8-way data parallelism over batch (one batch element per NeuronCore).

PE-row-minimal design (the cost model charges matmuls by moving-stream
length only; stationary loads are free):

  qT/kT : W-stationary projections, feature-major [e, t] outputs.
  v     : x^T-stationary projections, token-major [s, e] outputs with an
          all-ones column per head (-> softmax denominators ride the P@V).
  scores: per head, kT-slice stationary [64, 128s], qT moving -> [s, t]
          fp32 PSUM; exp on Act (PSUM->SBUF bf16); multiplicative
          exp(mask) factor on DVE.
  P@V   : probs-slice STATIONARY [128s, 128t], v moving [128s, 65]
          -> token-major [t, 65] PSUM accumulated over s-chunks:
          65 rows/matmul instead of 512.
  norm  : per-token-chunk LN stats (head-sum tree + elementwise square +
          accumulate, engine-steered: GpSimd at the phase boundary, DVE in
          the tail); rstd = exp(-0.5 ln(var+eps)) so every activation stays
          in the single preloaded natural_log_exp_and_others table set (no
          mid-kernel act-table reloads); DMA xbar transpose of the applied
          chunk to feature-major.
  out   : feature-major out-projection - stationary Wg slices (gamma
          folded), moving centered-attn^T; quarter-granular accumulation
          groups gate on individual chunk transposes. Output is [E, T]
          bf16 (host transposes and casts up).

Group schedule is HALF-MAJOR: (pair, half) groups run (0,0)..(7,0),
(0,1)..(7,1) so token chunks 0-3 complete at the phase midpoint and their
LN/transpose/out-projection overlap the half-1 compute; out-projection of
half 0 becomes PE filler for the Act-bound half-1 phase. q/k/v projections
fill phase A (weights and per-pair wv slices prefetched one slot ahead;
need-ordered startup DMA). Scores run three groups ahead of P@V (probs
quadruple-buffered; half-0 q tiles rotate through 3 buffers).
"""

from contextlib import ExitStack

import numpy as np
import ml_dtypes

import concourse.bass as bass
import concourse.mybir as mybir
from concourse import bacc, tile
from concourse.bass import ts
from concourse.bass_utils import run_bass_kernel_spmd

AF = mybir.ActivationFunctionType
ALU = mybir.AluOpType

B = 8
E = 1024
T = 1024
H = 16
HD = 64
P = 128
NCH = E // P          # feature chunks (= head pairs)
NTC = T // P          # token chunks
EPS = 1e-5
BF16 = mybir.dt.bfloat16
F32 = mybir.dt.float32
F32R = mybir.dt.float32r
NPBF16 = ml_dtypes.bfloat16


def _segs(lo, hi, split):
    """Token segments [lo, hi) split by modality boundary. -> [(s0, s1, m)]"""
    out = []
    if lo < min(hi, split):
        out.append((lo, min(hi, split), 0))
    if max(lo, split) < hi:
        out.append((max(lo, split), hi, 1))
    return out


def build_module(split: int, v_bias: bool, qk_bias: bool = True, o_bias: bool = True):
    assert 0 <= split <= T and split % 32 == 0, split
    nc = bacc.Bacc("TRN2", target_bir_lowering=False, debug=False)

    xqT = nc.declare_dram_parameter("xqT", [P, NCH * T], BF16, isOutput=False)
    xkT = nc.declare_dram_parameter("xkT", [P, NCH * T], BF16, isOutput=False)
    xvT = nc.declare_dram_parameter("xvT", [P, NCH * T], BF16, isOutput=False)
    wq = nc.declare_dram_parameter("wq", [2, NCH, P, NCH * P], BF16, isOutput=False)
    wk = nc.declare_dram_parameter("wk", [2, NCH, P, NCH * P], BF16, isOutput=False)
    wg = nc.declare_dram_parameter("wg", [2, 2, P, NCH * 512], BF16, isOutput=False)
    wv = nc.declare_dram_parameter("wv", [2, 2, P, NCH * 512], BF16, isOutput=False)
    em = nc.declare_dram_parameter("em", [P, NCH * T], BF16, isOutput=False)
    bq = nc.declare_dram_parameter("bq", [2, E], F32, isOutput=False)
    bk = nc.declare_dram_parameter("bk", [2, E], F32, isOutput=False)
    bv = nc.declare_dram_parameter("bv", [2, E], F32R, isOutput=False)
    c2 = nc.declare_dram_parameter("c2", [2, E], F32, isOutput=False)
    ones_d = nc.declare_dram_parameter("ones_d", [1, P], F32R, isOutput=False)
    outT = nc.declare_dram_parameter("outT", [E, T], BF16, isOutput=True)

    used_m = sorted(set(m for _, _, m in _segs(0, T, split)))

    with tile.TileContext(nc) as tc, ExitStack() as ctx:
        # preload the ln+exp activation-function set once: Exp, Ln, Square,
        # Identity and Copy all live in natural_log_exp_and_others (id 6),
        # so the exp stream never swaps tables mid-kernel
        nc.scalar.add_instruction(
            mybir.InstLoadActFuncSet(
                name=nc.get_next_instruction_name(),
                ins=[], outs=[], act_func_set_id=6,
            )
        )
        const = ctx.enter_context(tc.tile_pool(name="const", bufs=1))
        eps_t = const.tile([P, 1], F32)
        nc.vector.memset(eps_t[:], EPS)
        bq_sb = bk_sb = c2_sb = None
        if qk_bias:
            bq_sb = const.tile([P, 2 * NCH], F32)
            bk_sb = const.tile([P, 2 * NCH], F32)
            for m in (0, 1):
                cs = slice(m * NCH, (m + 1) * NCH)
                nc.sync.dma_start(bq_sb[:, cs], bq[m].rearrange("(c p) -> p c", p=P))
                nc.sync.dma_start(bk_sb[:, cs], bk[m].rearrange("(c p) -> p c", p=P))
        if o_bias:
            c2_sb = const.tile([P, 2 * NCH], F32)
            for m in (0, 1):
                cs = slice(m * NCH, (m + 1) * NCH)
                nc.sync.dma_start(c2_sb[:, cs], c2[m].rearrange("(c p) -> p c", p=P))
        ones_row = None
        bv_row_sb = None
        if v_bias:
            ones_row = const.tile([1, P], F32R)
            nc.sync.dma_start(ones_row[:], ones_d[:])
            bv_row_sb = const.tile([1, 2 * E], F32R)
            for m in (0, 1):
                nc.sync.dma_start(bv_row_sb[0:1, m * E:(m + 1) * E], bv[m][None, :])

        # ---------------- long-lived SBUF pools --------------------------
        # attn chunks rotate through 4 tags: chunk 4+j reuses chunk j's
        # buffer once chunk j's LN-apply (its last reader) is done.
        attn_pool = ctx.enter_context(tc.tile_pool(name="attn", bufs=1))
        attn_sc_box = {}

        def attn_sc(jj):
            if jj not in attn_sc_box:
                attn_sc_box[jj] = attn_pool.tile(
                    [P, E], BF16, tag=f"asc{jj % 4}", name=f"asc{jj}")
            return attn_sc_box[jj]

        attn_T3_box = {}

        stats_pool = ctx.enter_context(tc.tile_pool(name="stats", bufs=1))
        sums = [stats_pool.tile([P, H], F32, tag=f"sums{j}", name=f"sums{j}")
                for j in range(NTC)]
        scr_pool = ctx.enter_context(tc.tile_pool(name="scr", bufs=2))
        apl_pool = ctx.enter_context(tc.tile_pool(name="apl", bufs=2))
        em_pool = ctx.enter_context(tc.tile_pool(name="em", bufs=2))

        # PSUM pools
        proj_ps = ctx.enter_context(tc.tile_pool(name="proj_ps", bufs=2, space="PSUM"))
        sc_pool = ctx.enter_context(tc.tile_pool(name="sc_ps", bufs=2, space="PSUM"))
        at_pool = ctx.enter_context(tc.tile_pool(name="at_ps", bufs=2, space="PSUM"))

        main = ExitStack()
        with main:
            qk_sb = main.enter_context(tc.tile_pool(name="qk_sb", bufs=1))
            # half-0 q tiles die right after their scores group: rotate 3
            qk0_pool = main.enter_context(tc.tile_pool(name="qk0", bufs=3))
            vem_pool = main.enter_context(tc.tile_pool(name="vem", bufs=1))
            # probs pipeline: 4 generations deep (scores three groups ahead)
            pr_poolA = main.enter_context(tc.tile_pool(name="probsA", bufs=4))
            pr_poolB = main.enter_context(tc.tile_pool(name="probsB", bufs=4))
            wqk_pool = main.enter_context(tc.tile_pool(name="wqk", bufs=2))
            x_stack = ExitStack()
            x_pool = x_stack.enter_context(tc.tile_pool(name="xpool", bufs=1))

            wqk = {}

            def fetch_wqk(eo):
                # both modalities in one DMA (3D AP) to halve HWDGE holds
                for name, w_dram in (("q", wq), ("k", wk)):
                    if len(used_m) == 2:
                        wt = wqk_pool.tile([P, 2, NCH * P], BF16,
                                           tag=f"w{name}", name=f"w{name}e{eo}")
                        nc.sync.dma_start(
                            wt[:], w_dram[:, eo].rearrange("m p c -> p m c"))
                        for m in (0, 1):
                            wqk[(name, m, eo)] = wt[:, m]
                    else:
                        m = used_m[0]
                        wt = wqk_pool.tile([P, NCH * P], BF16,
                                           tag=f"w{name}", name=f"w{name}e{eo}")
                        nc.sync.dma_start(wt[:], w_dram[m, eo])
                        wqk[(name, m, eo)] = wt

            xq_tile = x_pool.tile([P, NCH * T], BF16, tag="xq", name="xq")
            xk_tile = x_pool.tile([P, NCH * T], BF16, tag="xk", name="xk")

            def dma_x_half(xt, xd, h0):
                for c in range(0, NCH, 2):
                    nc.sync.dma_start(xt[:, c * T:(c + 2) * T].rearrange(
                        "p (c2 t) -> p c2 t", t=T)[:, :, h0:h0 + 512],
                        xd[:, c * T:(c + 2) * T].rearrange(
                        "p (c2 t) -> p c2 t", t=T)[:, :, h0:h0 + 512])

            # em is staged one token-half at a time into half-size tiles
            # (double-buffered so half 1 can stream in early).
            em_box = {}

            def dma_em_half(h0):
                emt = em_pool.tile([P, NCH * 512], BF16, tag="em",
                                   name=f"em{h0}")
                em_box[1 if h0 else 0] = [
                    emt[:, c * 512:(c + 1) * 512] for c in range(NCH)]
                for c in range(0, NCH, 2):
                    nc.sync.dma_start(
                        emt[:, c * 512:(c + 2) * 512].rearrange(
                            "p (c2 t) -> p c2 t", t=512),
                        em[:, c * T:(c + 2) * T].rearrange(
                            "p (c2 t) -> p c2 t", t=T)[:, :, h0:h0 + 512])

            # startup DMAs, need-ordered and finely interleaved so the first
            # projection matmuls start as early as possible: wq(0) -> first
            # xq chunks -> wk(0) -> xk chunks -> mask half 0
            def fetch_w_one(name, w_dram, eo):
                # two half-DMAs: the projection's c-loop starts after the
                # first 4 feature chunks land
                if len(used_m) == 2:
                    wt = wqk_pool.tile([P, 2, NCH * P], BF16,
                                       tag=f"w{name}", name=f"w{name}e{eo}")
                    for h in (0, 1):
                        cs = slice(h * 512, (h + 1) * 512)
                        nc.sync.dma_start(
                            wt[:, :, cs],
                            w_dram[:, eo].rearrange("m p c -> p m c")[:, :, cs])
                    for m in (0, 1):
                        wqk[(name, m, eo)] = wt[:, m]
                else:
                    m = used_m[0]
                    wt = wqk_pool.tile([P, NCH * P], BF16,
                                       tag=f"w{name}", name=f"w{name}e{eo}")
                    for h in (0, 1):
                        cs = slice(h * 512, (h + 1) * 512)
                        nc.sync.dma_start(wt[:, cs], w_dram[m, eo][:, cs])
                    wqk[(name, m, eo)] = wt

            fetch_w_one("q", wq, 0)
            dma_x_half(xq_tile, xqT, 0)
            fetch_w_one("k", wk, 0)
            dma_x_half(xk_tile, xkT, 0)
            dma_x_half(xk_tile, xkT, 512)
            dma_em_half(0)
            fetch_wqk(1)
            dma_x_half(xq_tile, xqT, 512)

            xq_t = [xq_tile[:, c * T:(c + 1) * T] for c in range(NCH)]
            xk_t = [xk_tile[:, c * T:(c + 1) * T] for c in range(NCH)]

            # v output tiles (token-major, 66 cols per head, ones at col 64)
            v_t = []
            for tc_ in range(NTC):
                vt = vem_pool.tile([P, H * 65], BF16, tag=f"v{tc_}", name=f"v{tc_}")
                nc.vector.memset(
                    vt[:].rearrange("p (g w) -> p g w", w=65)[:, :, 64:65], 1.0
                )
                v_t.append(vt)

            qT_t, kT_t = {}, {}
            stats_tiles = {}

            def st(nm, jj):
                key = (nm, jj)
                if key not in stats_tiles:
                    stats_tiles[key] = stats_pool.tile(
                        [P, 1], F32, tag=f"{nm}{jj}", name=f"{nm}{jj}")
                return stats_tiles[key]

            def emit_qk_half(eo, half, name):
                x_t = xq_t if name == "q" else xk_t
                b_sb = bq_sb if name == "q" else bk_sb
                lo = half * 512
                if name == "q":
                    # q halves live in separate 512-col tiles; half 0
                    # rotates through 3 buffers (dead after its scores)
                    if half == 0:
                        qtile = qk0_pool.tile([P, 512], BF16, tag="qT0",
                                              name=f"qT0e{eo}")
                        qT_t[(eo, 0)] = qtile
                    else:
                        qtile = qk_sb.tile([P, 512], BF16, tag=f"qT1{eo}",
                                           name=f"qT1e{eo}")
                        qT_t[(eo, 1)] = qtile
                    toff = -lo  # tile-local columns
                else:
                    if eo not in kT_t:
                        kT_t[eo] = qk_sb.tile([P, T], BF16, tag=f"kT{eo}",
                                              name=f"kT{eo}")
                    qtile = kT_t[eo]
                    toff = 0
                ps = proj_ps.tile([P, 512], F32, tag="pp", name="pp")
                for s0, s1, m in _segs(lo, lo + 512, split):
                    wt = wqk[(name, m, eo)]
                    for c in range(NCH):
                        nc.tensor.matmul(
                            ps[:, s0 - lo:s1 - lo],
                            wt[:, ts(c, P)],
                            x_t[c][:, s0:s1],
                            start=(c == 0),
                            stop=(c == NCH - 1),
                        )
                # PSUM evacuation must stay off GpSimd (no PSUM access)
                if qk_bias:
                    for s0, s1, m in _segs(lo, lo + 512, split):
                        nc.vector.tensor_scalar_add(
                            qtile[:, s0 + toff:s1 + toff],
                            ps[:, s0 - lo:s1 - lo],
                            b_sb[:, m * NCH + eo:m * NCH + eo + 1],
                        )
                else:
                    nc.vector.tensor_copy(
                        qtile[:, lo + toff:lo + toff + 512], ps[:])

            # xv/wv in their own stack frame, closed after the v projection
            xvwv = ExitStack()
            xv_pool = xvwv.enter_context(tc.tile_pool(name="xv_p", bufs=1))
            xv_tile = xv_pool.tile([P, NCH * T], BF16, tag="xv", name="xv")
            wv_sb = {}

            def dma_xv():
                for c in range(0, NCH, 2):
                    nc.sync.dma_start(xv_tile[:, c * T:(c + 2) * T],
                                      xvT[:, c * T:(c + 2) * T])

            xv_t = [xv_tile[:, c * T:(c + 1) * T] for c in range(NCH)]

            def fetch_wv_pair(pair):
                # only the 128 e_out columns this head pair needs, strided
                # out of the eoh-block layout
                eoh, q4 = pair // 4, pair % 4
                for m in used_m:
                    wt = xv_pool.tile([P, NCH * P], BF16, tag=f"wv{m}",
                                      name=f"wv{m}p{pair}")
                    nc.sync.dma_start(
                        wt[:].rearrange("p (c w) -> p c w", w=P),
                        wv[m, eoh].rearrange("p (c w) -> p c w", w=512)[
                            :, :, q4 * P:(q4 + 1) * P])
                    wv_sb[(m, pair)] = wt

            def emit_v_pair(pair):
                """V projection for one head pair (128 e_out cols) over all
                token groups; 4 token groups share one PSUM tile so the
                evacuation amortizes and never gates the matmul stream."""
                eoh, q4 = pair // 4, pair % 4
                for tq in (0, 1):
                    ps = proj_ps.tile([P, 512], F32, tag="pp", name="pp")
                    for gi in range(4):
                        tc_ = 4 * tq + gi
                        lo = tc_ * P
                        gs = slice(gi * P, (gi + 1) * P)
                        for s0, s1, m in _segs(lo, lo + P, split):
                            m0, m1 = s0 - lo, s1 - lo
                            tp = (0, m0) if m0 else None
                            wvt = wv_sb[(m, pair)]
                            for c in range(NCH):
                                nc.tensor.matmul(
                                    ps[m0:m1, gs],
                                    xv_t[c][:, s0:s1],
                                    wvt[:, c * P:(c + 1) * P],
                                    start=(c == 0),
                                    stop=(c == NCH - 1) and not v_bias,
                                    tile_position=tp,
                                )
                            if v_bias:
                                nc.tensor.matmul(
                                    ps[m0:m1, gs],
                                    ones_row[0:1, 0:m1 - m0],
                                    bv_row_sb[
                                        0:1,
                                        m * E + eoh * 512 + q4 * P:
                                        m * E + eoh * 512 + (q4 + 1) * P,
                                    ].bitcast(F32R),
                                    start=False,
                                    stop=True,
                                    tile_position=tp,
                                )
                    for gi in range(4):
                        tc_ = 4 * tq + gi
                        dst = v_t[tc_][:].rearrange("p (g w) -> p g w", w=65)[
                            :, 2 * pair:2 * pair + 2, 0:64
                        ]
                        src_ = ps[:, gi * P:(gi + 1) * P].rearrange(
                            "p (g w) -> p g w", w=64)
                        nc.vector.tensor_copy(dst, src_)

            wg_sb = {}

            def fetch_wg():
                wg_pool = main.enter_context(tc.tile_pool(name="wg_sb", bufs=1))
                attn_T = wg_pool.tile([P, NCH * T], BF16, tag="attnT",
                                      name="attnT")
                attn_T3_box[0] = attn_T[:].rearrange("p (c t) -> p c t", t=T)
                for eoh in (0, 1):
                    for m in used_m:
                        wt = wg_pool.tile([P, NCH * 512], BF16,
                                          tag=f"wg{m}{eoh}", name=f"wg{m}{eoh}")
                        nc.sync.dma_start(wt[:], wg[m, eoh])
                        wg_sb[(m, eoh)] = wt

            def scores_c(pair, half, c, mask_on_pool=False):
                """Scores+exp+mask for one s-chunk of group (pair, half)."""
                lo = half * 512
                sc = sc_pool.tile([P, 1024], F32, tag="sc", name="sc")
                qsrc = qT_t[(pair, half)]
                nc.tensor.matmul(
                    sc[:, 0:512],
                    kT_t[pair][0:HD, ts(c, P)],
                    qsrc[0:HD, 0:512],
                )
                nc.tensor.matmul(
                    sc[:, 512:1024],
                    kT_t[pair][HD:P, ts(c, P)],
                    qsrc[HD:P, 0:512],
                )
                pr_pool = pr_poolA if c < 4 else pr_poolB
                pr = pr_pool.tile([P, 1024], BF16, tag=f"pr{c}", name=f"pr{c}")
                nc.scalar.activation(pr[:], sc[:], AF.Exp)
                emc = em_box[half][c]
                # the last two groups' early-chunk masks ride GpSimd (idle
                # after the half-0 LN chains); DVE is saturated there and
                # these masks have 2-3 slots of slack before their P@V
                pool_mask = mask_on_pool or (half == 1 and pair >= 5 and c < 4)
                eng = nc.gpsimd if pool_mask else nc.vector
                eng.tensor_mul(pr[:, 0:512], pr[:, 0:512], emc)
                eng.tensor_mul(pr[:, 512:1024], pr[:, 512:1024], emc)
                return pr

            def scores_group(pair, half):
                return [scores_c(pair, half, c) for c in range(NCH)]

            def chunk_stats(jj):
                """LN stats for token chunk jj: head sums -> mean; Act
                square (+fused accumulator for the tail chunks); per-chunk
                rstd and centering bias. For chunks 4-7 the small ops run
                on GpSimd/Act so the DVE stays on the P@V evac stream (the
                epilogue's pacing chain)."""
                tail = jj >= 4
                veng = nc.vector if tail else nc.gpsimd
                ap = apl_pool.tile([P, E], BF16, tag="apl", name=f"apl{jj}")
                stats_tiles[("apl", jj)] = ap
                # mean: one fused reduce over the 16 per-head sums (the
                # bf16 out scratch is the apply tile, overwritten next).
                # accum_out is DVE-only (invalid opcode on Pool).
                nc.vector.tensor_scalar(
                    ap[:, 0:H], sums[jj][:], 1.0, 0.0, ALU.mult, ALU.add,
                    accum_out=st("mus", jj)[:],
                )
                # square on Pool (boundary: Act+DVE saturated) / DVE (tail:
                # Act must not serialize the chunk chains); accumulate on DVE
                sq_eng = nc.vector if tail else nc.gpsimd
                sq_eng.tensor_tensor(ap[:], attn_sc(jj)[:], attn_sc(jj)[:],
                                     ALU.mult)
                nc.vector.tensor_scalar(
                    ap[:], ap[:], 1.0, 0.0, ALU.mult, ALU.add,
                    accum_out=st("sq", jj)[:],
                )
                mu2 = st("mu2", jj)
                veng.tensor_scalar(
                    mu2[:], st("mus", jj)[:], st("mus", jj)[:, 0:1],
                    1.0 / (E * E), ALU.mult, ALU.mult
                )
                var = st("var", jj)
                veng.tensor_scalar(
                    var[:], st("sq", jj)[:], 1.0 / E, mu2[:, 0:1],
                    ALU.mult, ALU.subtract
                )
                # rstd = exp(-0.5 ln(var+eps)): eps rides the Ln bias and
                # everything stays in the ln+exp act table set
                lnv = st("lnv", jj)
                nc.scalar.activation(lnv[:], var[:], AF.Ln,
                                     bias=eps_t[:, 0:1])
                rst = st("rst", jj)
                nc.scalar.activation(rst[:], lnv[:], AF.Exp, scale=-0.5)
                veng.tensor_scalar(
                    st("bsh", jj)[:], st("mus", jj)[:], rst[:, 0:1],
                    -1.0 / E, ALU.mult, ALU.mult
                )

            def apply_chunk(jj):
                """LN-apply for chunk jj. Chunks 0-3 run on GpSimd (Act and
                DVE are saturated at the phase boundary); chunks 4-7 run on
                DVE in 4x mode (idle in the tail, and Act must stay off the
                transpose-gating chain)."""
                ap = stats_tiles[("apl", jj)]
                eng = nc.vector if jj >= 4 else nc.gpsimd
                eng.tensor_scalar(
                    ap[:], attn_sc(jj)[:], st("rst", jj)[:, 0:1],
                    st("bsh", jj)[:, 0:1], ALU.mult, ALU.add,
                )

            def transpose_chunk(jj):
                nc.sync.dma_start_transpose(
                    attn_T3_box[0][:, :, jj * P:(jj + 1) * P],
                    stats_tiles[("apl", jj)][:],
                )

            def pv_j(pair, half, j, prs, with_stats=False):
                """One token-chunk of P@V for group (pair, half) with fused
                divide-normalize evacuation; on the last pair the LN stats +
                apply chain for chunk jj runs inline."""
                hA, hB = 2 * pair, 2 * pair + 1
                jj = half * 4 + j
                # both heads in one PSUM tile: one tag, double-buffered, so
                # the next pv_j's matmuls never wait on this one's evacs
                at = at_pool.tile([P, 130], F32, tag="at", name="at")
                for i, (h, toff) in enumerate(((hA, 0), (hB, 512))):
                    for c in range(NCH):
                        nc.tensor.matmul(
                            at[:, 65 * i:65 * i + 65],
                            prs[c][:, toff + j * P:toff + (j + 1) * P],
                            v_t[c][:, 65 * h:65 * h + 65],
                            start=(c == 0),
                            stop=(c == NCH - 1),
                        )
                den = stats_pool.tile([P, 2], F32, tag=f"den{j % 2}",
                                      name=f"den{j % 2}")
                for i in (0, 1):
                    nc.vector.reciprocal_approx_fast(
                        out=den[:, i:i + 1], in_=at[:, 65 * i + 64:65 * i + 65]
                    )
                for i, h in enumerate((hA, hB)):
                    nc.vector.tensor_scalar(
                        attn_sc(jj)[:, h * HD:(h + 1) * HD],
                        at[:, 65 * i:65 * i + HD],
                        den[:, i:i + 1],
                        0.0,
                        ALU.mult,
                        ALU.add,
                        accum_out=sums[jj][:, h:h + 1],
                    )
                if with_stats:
                    chunk_stats(jj)
                    apply_chunk(jj)
                    transpose_chunk(jj)

            def outproj_eo(half, eo, split_out=False):
                """Feature-major out-projection for one e_out chunk;
                quarter-granular accumulation groups gated on individual
                transposes. split_out pipelines the evacuation + store in
                halves (tail-drain)."""
                lo = half * 512
                ps = proj_ps.tile([P, 512], F32, tag="pp", name="pp")
                osb = scr_pool.tile([P, 512], BF16, tag="osb", name="osb")
                for q in range(4):
                    qlo = lo + q * P
                    for s0, s1, m in _segs(qlo, qlo + P, split):
                        wt = wg_sb[(m, eo // 4)]
                        for c in range(NCH):
                            nc.tensor.matmul(
                                ps[:, s0 - lo:s1 - lo],
                                wt[:].rearrange("p (c2 w) -> p c2 w", w=512)[
                                    :, c, (eo % 4) * P:(eo % 4 + 1) * P],
                                attn_T3_box[0][:, c, s0:s1],
                                start=(c == 0),
                                stop=(c == NCH - 1),
                            )
                    if split_out and q == 1:
                        nc.scalar.copy(osb[:, 0:256], ps[:, 0:256])
                        nc.sync.dma_start(
                            outT[ts(eo, P), lo:lo + 256], osb[:, 0:256])
                if o_bias:
                    for s0, s1, m in _segs(lo, lo + 512, split):
                        nc.scalar.activation(
                            osb[:, s0 - lo:s1 - lo], ps[:, s0 - lo:s1 - lo],
                            AF.Identity,
                            bias=c2_sb[:, m * NCH + eo:m * NCH + eo + 1],
                        )
                    nc.sync.dma_start(outT[ts(eo, P), lo:lo + 512], osb[:])
                elif split_out:
                    nc.scalar.copy(osb[:, 256:512], ps[:, 256:512])
                    nc.sync.dma_start(
                        outT[ts(eo, P), lo + 256:lo + 512], osb[:, 256:512])
                else:
                    if half == 0:
                        nc.vector.tensor_copy(osb[:], ps[:])
                    else:
                        # epilogue: Act idles once the exp stream has drained
                        nc.scalar.copy(osb[:], ps[:])
                    nc.sync.dma_start(outT[ts(eo, P), lo:lo + 512], osb[:])

            def pv_interleaved(pair, half, prs, nxt, fillers=(),
                               with_stats=False, mask_on_pool=False):
                """PV j-blocks for (pair, half) interleaved with the group
                two slots ahead's scores and PE filler work."""
                nxt_prs = []
                fillers = list(fillers)
                for j in range(4):
                    pv_j(pair, half, j, prs, with_stats=with_stats)
                    if fillers:
                        fillers.pop(0)()
                    if nxt is not None:
                        np_, nh = nxt
                        nxt_prs.append(scores_c(np_, nh, 2 * j, mask_on_pool))
                        nxt_prs.append(
                            scores_c(np_, nh, 2 * j + 1, mask_on_pool))
                for f in fillers:
                    f()
                return nxt_prs

            # ------------------------- group schedule ---------------------
            # prologue: full projections for pairs 0-2, scores for the first
            # three groups (the pipeline runs three groups ahead so the Act
            # exp stream never starves), v projection for pairs 0-1.
            prss = {}
            emit_qk_half(0, 0, "q")
            emit_qk_half(0, 0, "k")
            emit_qk_half(0, 1, "k")
            prss[(0, 0)] = [scores_c(0, 0, c) for c in range(4)]
            emit_qk_half(0, 1, "q")
            emit_qk_half(1, 0, "q")
            emit_qk_half(1, 0, "k")
            prss[(0, 0)] += [scores_c(0, 0, c) for c in range(4, 8)]
            emit_qk_half(1, 1, "k")
            emit_qk_half(1, 1, "q")
            dma_xv()
            fetch_wv_pair(0)
            fetch_wv_pair(1)
            prss[(1, 0)] = scores_group(1, 0)
            fetch_wqk(2)
            emit_qk_half(2, 0, "q")
            emit_qk_half(2, 0, "k")
            emit_v_pair(0)
            emit_qk_half(2, 1, "k")
            emit_qk_half(2, 1, "q")
            fetch_wqk(3)
            fetch_wv_pair(2)
            prss[(2, 0)] = scores_group(2, 0)
            emit_v_pair(1)
            dma_em_half(512)

            def qhs(eo):
                def f():
                    if eo + 1 < NCH:
                        fetch_wqk(eo + 1)
                    for half, name in ((0, "q"), (0, "k"), (1, "k"), (1, "q")):
                        emit_qk_half(eo, half, name)
                return f

            def vp(pair):
                def f():
                    emit_v_pair(pair)
                    if pair + 1 < NCH:
                        fetch_wv_pair(pair + 1)
                return f

            def op(half, eo):
                return lambda: outproj_eo(half, eo)

            def slot(p, h, np_, nh, fillers=(), with_stats=False,
                     mask_on_pool=False):
                nxt = (np_, nh) if np_ is not None else None
                nxt_prs = pv_interleaved(
                    p, h, prss.pop((p, h)), nxt, fillers,
                    with_stats=with_stats, mask_on_pool=mask_on_pool)
                if nxt is not None:
                    prss[nxt] = nxt_prs

            # ---- phase A: half-0 groups; q/k/v projections as filler ----
            # qhs(p) (as first filler) projects all four halves of pair p
            # right before that pair's scores and prefetches the next pair's
            # weights; vp(p) runs two slots ahead of its P@V group.
            slot(0, 0, 3, 0, [qhs(3), vp(2)])
            slot(1, 0, 4, 0, [qhs(4), vp(3)])
            slot(2, 0, 5, 0, [qhs(5), vp(4)])
            slot(3, 0, 6, 0, [qhs(6), vp(5)])
            slot(4, 0, 7, 0, [qhs(7), vp(6)])
            slot(5, 0, 0, 1, [vp(7)])
            xvwv.close()
            x_stack.close()
            fetch_wg()
            slot(6, 0, 1, 1)
            slot(7, 0, 2, 1)

            def chain(jj):
                def f():
                    chunk_stats(jj)
                    apply_chunk(jj)
                    transpose_chunk(jj)
                return f

            # ---- phase B: half-1 groups; the half-0 LN chains (emitted
            # here so they queue behind, not ahead of, the early half-1
            # mask/evac work), half-0 out-projection as filler.
            slot(0, 1, 3, 1, [chain(0), chain(1)])
            slot(1, 1, 4, 1, [chain(2), chain(3), op(0, 0)])
            slot(2, 1, 5, 1, [op(0, 1)])
            slot(3, 1, 6, 1, [op(0, 2)])
            slot(4, 1, 7, 1, [op(0, 3), op(0, 4)])
            slot(5, 1, None, None, [op(0, 5)])
            slot(6, 1, None, None, [op(0, 6)])
            outproj_eo(0, 7)

            # ---- epilogue: last group with inline stats+apply+transpose;
            # the half-1 out-projections' quarter accumulation groups gate
            # on the individual transposes, so emitting them after the last
            # transpose still overlaps the chunk 4-6 chains.
            prs71 = prss.pop((7, 1))
            pv_j(7, 1, 0, prs71, with_stats=True)
            pv_j(7, 1, 1, prs71, with_stats=True)
            pv_j(7, 1, 2, prs71, with_stats=True)
            pv_j(7, 1, 3, prs71, with_stats=True)
            for eo in range(NCH):
                outproj_eo(1, eo, split_out=(eo >= NCH - 2))

    nc.compile()
    return nc


def _pack_pmajor(arr2d):
    # [NCH*P, T] -> [P, NCH*T]: row p holds chunk-major concatenation
    return np.ascontiguousarray(
        arr2d.reshape(NCH, P, T).transpose(1, 0, 2).reshape(P, NCH * T)
    )


def _pack_wv_style(Wt, Wi):
    # [2, eoh, p, c*512+j] with arr[c*128+p, eoh*512+j], arr = W.T
    out = np.empty((2, 2, P, NCH * 512), NPBF16)
    for m, W in enumerate((Wt, Wi)):
        arr = (W.T).astype(NPBF16)  # [e_in, e_out]
        out[m] = (
            arr.reshape(NCH, P, 2, 512)
            .transpose(2, 1, 0, 3)
            .reshape(2, P, NCH * 512)
        )
    return np.ascontiguousarray(out)


def _host_prep(inputs):
    scaling = HD ** -0.5
    f32 = np.float32

    def a(name):
        return np.asarray(inputs[name], f32)

    def prep_blocks(Wt, Wi, scale=1.0):
        # [2, eo, p, c*128+j] with arr[c*128+p, eo*128+j]
        out = np.empty((2, NCH, P, NCH * P), NPBF16)
        for m, W in enumerate((Wt, Wi)):
            arr = ((W * scale).T).astype(NPBF16)  # [e_in, e_out]
            out[m] = (
                arr.reshape(NCH, P, NCH, P)
                .transpose(2, 1, 0, 3)
                .reshape(NCH, P, NCH * P)
            )
        return np.ascontiguousarray(out)

    Wo_t, Wo_i = a("Wo_t"), a("Wo_i")
    g_t, g_i = a("ln_g_t"), a("ln_g_i")
    b_t, b_i = a("ln_b_t"), a("ln_b_i")
    Wg_t = Wo_t * g_t[None, :]
    Wg_i = Wo_i * g_i[None, :]

    wq_np = prep_blocks(a("Wq_t"), a("Wq_i"), scaling)
    wk_np = prep_blocks(a("Wk_t"), a("Wk_i"))
    wg_np = _pack_wv_style(Wg_t, Wg_i)
    wv_np = _pack_wv_style(a("Wv_t"), a("Wv_i"))

    em_np = _pack_pmajor(
        np.exp(np.asarray(inputs["attention_mask"], np.float64)).T.astype(NPBF16)
    )

    bq_np = np.stack([a("bq_t"), a("bq_i")]) * f32(scaling)
    bk_np = np.stack([a("bk_t"), a("bk_i")])
    bv_np = np.stack([a("bv_t"), a("bv_i")])
    c2_np = np.stack(
        [
            Wo_t.astype(np.float64) @ b_t.astype(np.float64) + a("bo_t"),
            Wo_i.astype(np.float64) @ b_i.astype(np.float64) + a("bo_i"),
        ]
    ).astype(f32)
    ones_np = np.ones((1, P), np.float32)

    shared = dict(
        wq=wq_np, wk=wk_np, wg=wg_np, wv=wv_np, em=em_np,
        ones_d=ones_np,
        bq=np.ascontiguousarray(bq_np), bk=np.ascontiguousarray(bk_np),
        bv=np.ascontiguousarray(bv_np), c2=np.ascontiguousarray(c2_np),
    )
    flags = (
        bool(np.any(bv_np)),
        bool(np.any(bq_np) or np.any(bk_np)),
        bool(np.any(c2_np)),
    )
    return shared, flags


_CACHE = {}


def build_cached(split, flags):
    key = (split, flags)
    if key not in _CACHE:
        _CACHE[key] = build_module(split, *flags)
    return _CACHE[key]


def kernel(**inputs):
    q = np.asarray(inputs["query"], np.float32)
    k = np.asarray(inputs["key"], np.float32)
    v = np.asarray(inputs["value"], np.float32)
    assert q.shape == (B, T, E), q.shape
    split = int(np.asarray(inputs["split_position"]))

    shared, flags = _host_prep(inputs)
    nc = build_cached(split, flags)

    in_maps = []
    for b in range(B):
        m = dict(shared)
        m["xqT"] = _pack_pmajor(q[b].T.astype(NPBF16))
        m["xkT"] = _pack_pmajor(k[b].T.astype(NPBF16))
        m["xvT"] = _pack_pmajor(v[b].T.astype(NPBF16))
        in_maps.append(m)

    res = run_bass_kernel_spmd(nc, in_maps, list(range(B)))
    out = np.stack(
        [np.ascontiguousarray(res.results[b]["outT"].T) for b in range(B)]
    )
    return out.astype(np.float32)


# revision 79
# speedup vs baseline: 1.0251x; 1.0099x over previous
"""Trainium2 Bass kernel for BEiT-3 multiway multihead attention, v3.

Strategy
--------
8-way data parallelism over batch (one batch element per NeuronCore).

PE-row-minimal design (the cost model charges matmuls by moving-stream
length only; stationary loads are free):

  qT/kT : W-stationary projections, feature-major [e, t] outputs.
  v     : x^T-stationary projections, token-major [s, e] outputs with an
          all-ones column per head (-> softmax denominators ride the P@V).
  scores: per head, kT-slice stationary [64, 128s], qT moving -> [s, t]
          fp32 PSUM; exp on Act (PSUM->SBUF bf16); multiplicative
          exp(mask) factor on DVE.
  P@V   : probs-slice STATIONARY [128s, 128t], v moving [128s, 65]
          -> token-major [t, 65] PSUM accumulated over s-chunks:
          65 rows/matmul instead of 512.
  norm  : per-token-chunk LN stats (head-sum tree + elementwise square +
          accumulate, engine-steered: GpSimd at the phase boundary, DVE in
          the tail); rstd = exp(-0.5 ln(var+eps)) so every activation stays
          in the single preloaded natural_log_exp_and_others table set (no
          mid-kernel act-table reloads); DMA xbar transpose of the applied
          chunk to feature-major.
  out   : feature-major out-projection - stationary Wg slices (gamma
          folded), moving centered-attn^T; quarter-granular accumulation
          groups gate on individual chunk transposes. Output is [E, T]
          bf16 (host transposes and casts up).

Group schedule is HALF-MAJOR: (pair, half) groups run (0,0)..(7,0),
(0,1)..(7,1) so token chunks 0-3 complete at the phase midpoint and their
LN/transpose/out-projection overlap the half-1 compute; out-projection of
half 0 becomes PE filler for the Act-bound half-1 phase. q/k/v projections
fill phase A (weights and per-pair wv slices prefetched one slot ahead;
need-ordered startup DMA). Scores run three groups ahead of P@V (probs
quadruple-buffered; half-0 q tiles rotate through 3 buffers).
"""

from contextlib import ExitStack

import numpy as np
import ml_dtypes

import concourse.bass as bass
import concourse.mybir as mybir
from concourse import bacc, tile
from concourse.bass import ts
from concourse.bass_utils import run_bass_kernel_spmd

AF = mybir.ActivationFunctionType
ALU = mybir.AluOpType

B = 8
E = 1024
T = 1024
H = 16
HD = 64
P = 128
NCH = E // P          # feature chunks (= head pairs)
NTC = T // P          # token chunks
EPS = 1e-5
BF16 = mybir.dt.bfloat16
F32 = mybir.dt.float32
F32R = mybir.dt.float32r
NPBF16 = ml_dtypes.bfloat16


def _segs(lo, hi, split):
    """Token segments [lo, hi) split by modality boundary. -> [(s0, s1, m)]"""
    out = []
    if lo < min(hi, split):
        out.append((lo, min(hi, split), 0))
    if max(lo, split) < hi:
        out.append((max(lo, split), hi, 1))
    return out


def build_module(split: int, v_bias: bool, qk_bias: bool = True, o_bias: bool = True):
    assert 0 <= split <= T and split % 32 == 0, split
    nc = bacc.Bacc("TRN2", target_bir_lowering=False, debug=False)

    xqT = nc.declare_dram_parameter("xqT", [P, NCH * T], BF16, isOutput=False)
    xkT = nc.declare_dram_parameter("xkT", [P, NCH * T], BF16, isOutput=False)
    xvT = nc.declare_dram_parameter("xvT", [P, NCH * T], BF16, isOutput=False)
    wq = nc.declare_dram_parameter("wq", [2, NCH, P, NCH * P], BF16, isOutput=False)
    wk = nc.declare_dram_parameter("wk", [2, NCH, P, NCH * P], BF16, isOutput=False)
    wg = nc.declare_dram_parameter("wg", [2, 2, P, NCH * 512], BF16, isOutput=False)
    wv = nc.declare_dram_parameter("wv", [2, 2, P, NCH * 512], BF16, isOutput=False)
    em = nc.declare_dram_parameter("em", [P, NCH * T], BF16, isOutput=False)
    bq = nc.declare_dram_parameter("bq", [2, E], F32, isOutput=False)
    bk = nc.declare_dram_parameter("bk", [2, E], F32, isOutput=False)
    bv = nc.declare_dram_parameter("bv", [2, E], F32R, isOutput=False)
    c2 = nc.declare_dram_parameter("c2", [2, E], F32, isOutput=False)
    ones_d = nc.declare_dram_parameter("ones_d", [1, P], F32R, isOutput=False)
    outT = nc.declare_dram_parameter("outT", [E, T], BF16, isOutput=True)

    used_m = sorted(set(m for _, _, m in _segs(0, T, split)))

    with tile.TileContext(nc) as tc, ExitStack() as ctx:
        # preload the ln+exp activation-function set once: Exp, Ln, Square,
        # Identity and Copy all live in natural_log_exp_and_others (id 6),
        # so the exp stream never swaps tables mid-kernel
        nc.scalar.add_instruction(
            mybir.InstLoadActFuncSet(
                name=nc.get_next_instruction_name(),
                ins=[], outs=[], act_func_set_id=6,
            )
        )
        const = ctx.enter_context(tc.tile_pool(name="const", bufs=1))
        eps_t = const.tile([P, 1], F32)
        nc.vector.memset(eps_t[:], EPS)
        bq_sb = bk_sb = c2_sb = None
        if qk_bias:
            bq_sb = const.tile([P, 2 * NCH], F32)
            bk_sb = const.tile([P, 2 * NCH], F32)
            for m in (0, 1):
                cs = slice(m * NCH, (m + 1) * NCH)
                nc.sync.dma_start(bq_sb[:, cs], bq[m].rearrange("(c p) -> p c", p=P))
                nc.sync.dma_start(bk_sb[:, cs], bk[m].rearrange("(c p) -> p c", p=P))
        if o_bias:
            c2_sb = const.tile([P, 2 * NCH], F32)
            for m in (0, 1):
                cs = slice(m * NCH, (m + 1) * NCH)
                nc.sync.dma_start(c2_sb[:, cs], c2[m].rearrange("(c p) -> p c", p=P))
        ones_row = None
        bv_row_sb = None
        if v_bias:
            ones_row = const.tile([1, P], F32R)
            nc.sync.dma_start(ones_row[:], ones_d[:])
            bv_row_sb = const.tile([1, 2 * E], F32R)
            for m in (0, 1):
                nc.sync.dma_start(bv_row_sb[0:1, m * E:(m + 1) * E], bv[m][None, :])

        # ---------------- long-lived SBUF pools --------------------------
        # attn chunks rotate through 4 tags: chunk 4+j reuses chunk j's
        # buffer once chunk j's LN-apply (its last reader) is done.
        attn_pool = ctx.enter_context(tc.tile_pool(name="attn", bufs=1))
        attn_sc_box = {}

        def attn_sc(jj):
            if jj not in attn_sc_box:
                attn_sc_box[jj] = attn_pool.tile(
                    [P, E], BF16, tag=f"asc{jj % 4}", name=f"asc{jj}")
            return attn_sc_box[jj]

        attn_T3_box = {}

        stats_pool = ctx.enter_context(tc.tile_pool(name="stats", bufs=1))
        sums = [stats_pool.tile([P, H], F32, tag=f"sums{j}", name=f"sums{j}")
                for j in range(NTC)]
        scr_pool = ctx.enter_context(tc.tile_pool(name="scr", bufs=2))
        apl_pool = ctx.enter_context(tc.tile_pool(name="apl", bufs=2))
        em_pool = ctx.enter_context(tc.tile_pool(name="em", bufs=2))

        # PSUM pools
        proj_ps = ctx.enter_context(tc.tile_pool(name="proj_ps", bufs=2, space="PSUM"))
        sc_pool = ctx.enter_context(tc.tile_pool(name="sc_ps", bufs=2, space="PSUM"))
        at_pool = ctx.enter_context(tc.tile_pool(name="at_ps", bufs=2, space="PSUM"))

        main = ExitStack()
        with main:
            qk_sb = main.enter_context(tc.tile_pool(name="qk_sb", bufs=1))
            # half-0 q tiles die right after their scores group: rotate 3
            qk0_pool = main.enter_context(tc.tile_pool(name="qk0", bufs=3))
            vem_pool = main.enter_context(tc.tile_pool(name="vem", bufs=1))
            # probs pipeline: 4 generations deep (scores three groups ahead)
            pr_poolA = main.enter_context(tc.tile_pool(name="probsA", bufs=4))
            pr_poolB = main.enter_context(tc.tile_pool(name="probsB", bufs=4))
            wqk_pool = main.enter_context(tc.tile_pool(name="wqk", bufs=2))
            x_stack = ExitStack()
            x_pool = x_stack.enter_context(tc.tile_pool(name="xpool", bufs=1))

            wqk = {}

            def fetch_wqk(eo):
                # both modalities in one DMA (3D AP) to halve HWDGE holds
                for name, w_dram in (("q", wq), ("k", wk)):
                    if len(used_m) == 2:
                        wt = wqk_pool.tile([P, 2, NCH * P], BF16,
                                           tag=f"w{name}", name=f"w{name}e{eo}")
                        nc.sync.dma_start(
                            wt[:], w_dram[:, eo].rearrange("m p c -> p m c"))
                        for m in (0, 1):
                            wqk[(name, m, eo)] = wt[:, m]
                    else:
                        m = used_m[0]
                        wt = wqk_pool.tile([P, NCH * P], BF16,
                                           tag=f"w{name}", name=f"w{name}e{eo}")
                        nc.sync.dma_start(wt[:], w_dram[m, eo])
                        wqk[(name, m, eo)] = wt

            xq_tile = x_pool.tile([P, NCH * T], BF16, tag="xq", name="xq")
            xk_tile = x_pool.tile([P, NCH * T], BF16, tag="xk", name="xk")

            def dma_x_half(xt, xd, h0):
                for c in range(0, NCH, 2):
                    nc.sync.dma_start(xt[:, c * T:(c + 2) * T].rearrange(
                        "p (c2 t) -> p c2 t", t=T)[:, :, h0:h0 + 512],
                        xd[:, c * T:(c + 2) * T].rearrange(
                        "p (c2 t) -> p c2 t", t=T)[:, :, h0:h0 + 512])

            # em is staged one token-half at a time into half-size tiles
            # (double-buffered so half 1 can stream in early).
            em_box = {}

            def dma_em_half(h0):
                emt = em_pool.tile([P, NCH * 512], BF16, tag="em",
                                   name=f"em{h0}")
                em_box[1 if h0 else 0] = [
                    emt[:, c * 512:(c + 1) * 512] for c in range(NCH)]
                for c in range(0, NCH, 2):
                    nc.sync.dma_start(
                        emt[:, c * 512:(c + 2) * 512].rearrange(
                            "p (c2 t) -> p c2 t", t=512),
                        em[:, c * T:(c + 2) * T].rearrange(
                            "p (c2 t) -> p c2 t", t=T)[:, :, h0:h0 + 512])

            # startup DMAs, need-ordered and finely interleaved so the first
            # projection matmuls start as early as possible: wq(0) -> first
            # xq chunks -> wk(0) -> xk chunks -> mask half 0
            def fetch_w_one(name, w_dram, eo):
                # two half-DMAs: the projection's c-loop starts after the
                # first 4 feature chunks land
                if len(used_m) == 2:
                    wt = wqk_pool.tile([P, 2, NCH * P], BF16,
                                       tag=f"w{name}", name=f"w{name}e{eo}")
                    for h in (0, 1):
                        cs = slice(h * 512, (h + 1) * 512)
                        nc.sync.dma_start(
                            wt[:, :, cs],
                            w_dram[:, eo].rearrange("m p c -> p m c")[:, :, cs])
                    for m in (0, 1):
                        wqk[(name, m, eo)] = wt[:, m]
                else:
                    m = used_m[0]
                    wt = wqk_pool.tile([P, NCH * P], BF16,
                                       tag=f"w{name}", name=f"w{name}e{eo}")
                    for h in (0, 1):
                        cs = slice(h * 512, (h + 1) * 512)
                        nc.sync.dma_start(wt[:, cs], w_dram[m, eo][:, cs])
                    wqk[(name, m, eo)] = wt

            fetch_w_one("q", wq, 0)
            dma_x_half(xq_tile, xqT, 0)
            fetch_w_one("k", wk, 0)
            dma_x_half(xk_tile, xkT, 0)
            dma_x_half(xk_tile, xkT, 512)
            dma_em_half(0)
            fetch_wqk(1)
            dma_x_half(xq_tile, xqT, 512)

            xq_t = [xq_tile[:, c * T:(c + 1) * T] for c in range(NCH)]
            xk_t = [xk_tile[:, c * T:(c + 1) * T] for c in range(NCH)]

            # v output tiles (token-major, 66 cols per head, ones at col 64)
            v_t = []
            for tc_ in range(NTC):
                vt = vem_pool.tile([P, H * 65], BF16, tag=f"v{tc_}", name=f"v{tc_}")
                nc.vector.memset(
                    vt[:].rearrange("p (g w) -> p g w", w=65)[:, :, 64:65], 1.0
                )
                v_t.append(vt)

            qT_t, kT_t = {}, {}
            stats_tiles = {}

            def st(nm, jj):
                key = (nm, jj)
                if key not in stats_tiles:
                    stats_tiles[key] = stats_pool.tile(
                        [P, 1], F32, tag=f"{nm}{jj}", name=f"{nm}{jj}")
                return stats_tiles[key]

            def emit_qk_half(eo, half, name):
                x_t = xq_t if name == "q" else xk_t
                b_sb = bq_sb if name == "q" else bk_sb
                lo = half * 512
                if name == "q":
                    # q halves live in separate 512-col tiles; half 0
                    # rotates through 3 buffers (dead after its scores)
                    if half == 0:
                        qtile = qk0_pool.tile([P, 512], BF16, tag="qT0",
                                              name=f"qT0e{eo}")
                        qT_t[(eo, 0)] = qtile
                    else:
                        qtile = qk_sb.tile([P, 512], BF16, tag=f"qT1{eo}",
                                           name=f"qT1e{eo}")
                        qT_t[(eo, 1)] = qtile
                    toff = -lo  # tile-local columns
                else:
                    if eo not in kT_t:
                        kT_t[eo] = qk_sb.tile([P, T], BF16, tag=f"kT{eo}",
                                              name=f"kT{eo}")
                    qtile = kT_t[eo]
                    toff = 0
                ps = proj_ps.tile([P, 512], F32, tag="pp", name="pp")
                for s0, s1, m in _segs(lo, lo + 512, split):
                    wt = wqk[(name, m, eo)]
                    for c in range(NCH):
                        nc.tensor.matmul(
                            ps[:, s0 - lo:s1 - lo],
                            wt[:, ts(c, P)],
                            x_t[c][:, s0:s1],
                            start=(c == 0),
                            stop=(c == NCH - 1),
                        )
                # PSUM evacuation must stay off GpSimd (no PSUM access)
                if qk_bias:
                    for s0, s1, m in _segs(lo, lo + 512, split):
                        nc.vector.tensor_scalar_add(
                            qtile[:, s0 + toff:s1 + toff],
                            ps[:, s0 - lo:s1 - lo],
                            b_sb[:, m * NCH + eo:m * NCH + eo + 1],
                        )
                else:
                    nc.vector.tensor_copy(
                        qtile[:, lo + toff:lo + toff + 512], ps[:])

            # xv/wv in their own stack frame, closed after the v projection
            xvwv = ExitStack()
            xv_pool = xvwv.enter_context(tc.tile_pool(name="xv_p", bufs=1))
            xv_tile = xv_pool.tile([P, NCH * T], BF16, tag="xv", name="xv")
            wv_sb = {}

            def dma_xv():
                for c in range(0, NCH, 2):
                    nc.sync.dma_start(xv_tile[:, c * T:(c + 2) * T],
                                      xvT[:, c * T:(c + 2) * T])

            xv_t = [xv_tile[:, c * T:(c + 1) * T] for c in range(NCH)]

            def fetch_wv_pair(pair):
                # only the 128 e_out columns this head pair needs, strided
                # out of the eoh-block layout
                eoh, q4 = pair // 4, pair % 4
                for m in used_m:
                    wt = xv_pool.tile([P, NCH * P], BF16, tag=f"wv{m}",
                                      name=f"wv{m}p{pair}")
                    nc.sync.dma_start(
                        wt[:].rearrange("p (c w) -> p c w", w=P),
                        wv[m, eoh].rearrange("p (c w) -> p c w", w=512)[
                            :, :, q4 * P:(q4 + 1) * P])
                    wv_sb[(m, pair)] = wt

            def emit_v_pair(pair):
                """V projection for one head pair (128 e_out cols) over all
                token groups; 4 token groups share one PSUM tile so the
                evacuation amortizes and never gates the matmul stream."""
                eoh, q4 = pair // 4, pair % 4
                for tq in (0, 1):
                    ps = proj_ps.tile([P, 512], F32, tag="pp", name="pp")
                    for gi in range(4):
                        tc_ = 4 * tq + gi
                        lo = tc_ * P
                        gs = slice(gi * P, (gi + 1) * P)
                        for s0, s1, m in _segs(lo, lo + P, split):
                            m0, m1 = s0 - lo, s1 - lo
                            tp = (0, m0) if m0 else None
                            wvt = wv_sb[(m, pair)]
                            for c in range(NCH):
                                nc.tensor.matmul(
                                    ps[m0:m1, gs],
                                    xv_t[c][:, s0:s1],
                                    wvt[:, c * P:(c + 1) * P],
                                    start=(c == 0),
                                    stop=(c == NCH - 1) and not v_bias,
                                    tile_position=tp,
                                )
                            if v_bias:
                                nc.tensor.matmul(
                                    ps[m0:m1, gs],
                                    ones_row[0:1, 0:m1 - m0],
                                    bv_row_sb[
                                        0:1,
                                        m * E + eoh * 512 + q4 * P:
                                        m * E + eoh * 512 + (q4 + 1) * P,
                                    ].bitcast(F32R),
                                    start=False,
                                    stop=True,
                                    tile_position=tp,
                                )
                    for gi in range(4):
                        tc_ = 4 * tq + gi
                        dst = v_t[tc_][:].rearrange("p (g w) -> p g w", w=65)[
                            :, 2 * pair:2 * pair + 2, 0:64
                        ]
                        src_ = ps[:, gi * P:(gi + 1) * P].rearrange(
                            "p (g w) -> p g w", w=64)
                        nc.vector.tensor_copy(dst, src_)

            wg_sb = {}

            def fetch_wg():
                wg_pool = main.enter_context(tc.tile_pool(name="wg_sb", bufs=1))
                attn_T = wg_pool.tile([P, NCH * T], BF16, tag="attnT",
                                      name="attnT")
                attn_T3_box[0] = attn_T[:].rearrange("p (c t) -> p c t", t=T)
                for eoh in (0, 1):
                    for m in used_m:
                        wt = wg_pool.tile([P, NCH * 512], BF16,
                                          tag=f"wg{m}{eoh}", name=f"wg{m}{eoh}")
                        nc.sync.dma_start(wt[:], wg[m, eoh])
                        wg_sb[(m, eoh)] = wt

            def scores_c(pair, half, c, mask_on_pool=False):
                """Scores+exp+mask for one s-chunk of group (pair, half)."""
                lo = half * 512
                sc = sc_pool.tile([P, 1024], F32, tag="sc", name="sc")
                qsrc = qT_t[(pair, half)]
                nc.tensor.matmul(
                    sc[:, 0:512],
                    kT_t[pair][0:HD, ts(c, P)],
                    qsrc[0:HD, 0:512],
                )
                nc.tensor.matmul(
                    sc[:, 512:1024],
                    kT_t[pair][HD:P, ts(c, P)],
                    qsrc[HD:P, 0:512],
                )
                pr_pool = pr_poolA if c < 4 else pr_poolB
                pr = pr_pool.tile([P, 1024], BF16, tag=f"pr{c}", name=f"pr{c}")
                nc.scalar.activation(pr[:], sc[:], AF.Exp)
                emc = em_box[half][c]
                # the last two groups' early-chunk masks ride GpSimd (idle
                # after the half-0 LN chains); DVE is saturated there and
                # these masks have 2-3 slots of slack before their P@V
                pool_mask = mask_on_pool or (half == 1 and pair >= 5 and c < 4)
                eng = nc.gpsimd if pool_mask else nc.vector
                eng.tensor_mul(pr[:, 0:512], pr[:, 0:512], emc)
                eng.tensor_mul(pr[:, 512:1024], pr[:, 512:1024], emc)
                return pr

            def scores_group(pair, half):
                return [scores_c(pair, half, c) for c in range(NCH)]

            def chunk_stats(jj):
                """LN stats for token chunk jj: head sums -> mean; Act
                square (+fused accumulator for the tail chunks); per-chunk
                rstd and centering bias. For chunks 4-7 the small ops run
                on GpSimd/Act so the DVE stays on the P@V evac stream (the
                epilogue's pacing chain)."""
                tail = jj >= 4
                veng = nc.vector if tail else nc.gpsimd
                ap = apl_pool.tile([P, E], BF16, tag="apl", name=f"apl{jj}")
                stats_tiles[("apl", jj)] = ap
                # mean: one fused reduce over the 16 per-head sums (the
                # bf16 out scratch is the apply tile, overwritten next).
                # accum_out is DVE-only (invalid opcode on Pool).
                nc.vector.tensor_scalar(
                    ap[:, 0:H], sums[jj][:], 1.0, 0.0, ALU.mult, ALU.add,
                    accum_out=st("mus", jj)[:],
                )
                # square on Pool (boundary: Act+DVE saturated) / DVE (tail:
                # Act must not serialize the chunk chains); accumulate on DVE
                sq_eng = nc.vector if tail else nc.gpsimd
                sq_eng.tensor_tensor(ap[:], attn_sc(jj)[:], attn_sc(jj)[:],
                                     ALU.mult)
                nc.vector.tensor_scalar(
                    ap[:], ap[:], 1.0, 0.0, ALU.mult, ALU.add,
                    accum_out=st("sq", jj)[:],
                )
                mu2 = st("mu2", jj)
                veng.tensor_scalar(
                    mu2[:], st("mus", jj)[:], st("mus", jj)[:, 0:1],
                    1.0 / (E * E), ALU.mult, ALU.mult
                )
                var = st("var", jj)
                veng.tensor_scalar(
                    var[:], st("sq", jj)[:], 1.0 / E, mu2[:, 0:1],
                    ALU.mult, ALU.subtract
                )
                # rstd = exp(-0.5 ln(var+eps)): eps rides the Ln bias and
                # everything stays in the ln+exp act table set
                lnv = st("lnv", jj)
                nc.scalar.activation(lnv[:], var[:], AF.Ln,
                                     bias=eps_t[:, 0:1])
                rst = st("rst", jj)
                nc.scalar.activation(rst[:], lnv[:], AF.Exp, scale=-0.5)
                veng.tensor_scalar(
                    st("bsh", jj)[:], st("mus", jj)[:], rst[:, 0:1],
                    -1.0 / E, ALU.mult, ALU.mult
                )

            def apply_chunk(jj):
                """LN-apply for chunk jj. Chunks 0-3 run on GpSimd (Act and
                DVE are saturated at the phase boundary); chunks 4-7 run on
                DVE in 4x mode (idle in the tail, and Act must stay off the
                transpose-gating chain)."""
                ap = stats_tiles[("apl", jj)]
                eng = nc.vector if jj >= 4 else nc.gpsimd
                eng.tensor_scalar(
                    ap[:], attn_sc(jj)[:], st("rst", jj)[:, 0:1],
                    st("bsh", jj)[:, 0:1], ALU.mult, ALU.add,
                )

            def transpose_chunk(jj):
                nc.sync.dma_start_transpose(
                    attn_T3_box[0][:, :, jj * P:(jj + 1) * P],
                    stats_tiles[("apl", jj)][:],
                )

            def pv_j(pair, half, j, prs, with_stats=False):
                """One token-chunk of P@V for group (pair, half) with fused
                divide-normalize evacuation; on the last pair the LN stats +
                apply chain for chunk jj runs inline."""
                hA, hB = 2 * pair, 2 * pair + 1
                jj = half * 4 + j
                # both heads in one PSUM tile: one tag, double-buffered, so
                # the next pv_j's matmuls never wait on this one's evacs
                at = at_pool.tile([P, 130], F32, tag="at", name="at")
                for i, (h, toff) in enumerate(((hA, 0), (hB, 512))):
                    for c in range(NCH):
                        nc.tensor.matmul(
                            at[:, 65 * i:65 * i + 65],
                            prs[c][:, toff + j * P:toff + (j + 1) * P],
                            v_t[c][:, 65 * h:65 * h + 65],
                            start=(c == 0),
                            stop=(c == NCH - 1),
                        )
                den = stats_pool.tile([P, 2], F32, tag=f"den{j % 2}",
                                      name=f"den{j % 2}")
                for i in (0, 1):
                    nc.vector.reciprocal_approx_fast(
                        out=den[:, i:i + 1], in_=at[:, 65 * i + 64:65 * i + 65]
                    )
                for i, h in enumerate((hA, hB)):
                    nc.vector.tensor_scalar(
                        attn_sc(jj)[:, h * HD:(h + 1) * HD],
                        at[:, 65 * i:65 * i + HD],
                        den[:, i:i + 1],
                        0.0,
                        ALU.mult,
                        ALU.add,
                        accum_out=sums[jj][:, h:h + 1],
                    )
                if with_stats:
                    chunk_stats(jj)
                    apply_chunk(jj)
                    transpose_chunk(jj)

            def outproj_eo(half, eo, split_out=False):
                """Feature-major out-projection for one e_out chunk;
                quarter-granular accumulation groups gated on individual
                transposes. split_out pipelines the evacuation + store in
                halves (tail-drain)."""
                lo = half * 512
                ps = proj_ps.tile([P, 512], F32, tag="pp", name="pp")
                osb = scr_pool.tile([P, 512], BF16, tag="osb", name="osb")
                for q in range(4):
                    qlo = lo + q * P
                    for s0, s1, m in _segs(qlo, qlo + P, split):
                        wt = wg_sb[(m, eo // 4)]
                        for c in range(NCH):
                            nc.tensor.matmul(
                                ps[:, s0 - lo:s1 - lo],
                                wt[:].rearrange("p (c2 w) -> p c2 w", w=512)[
                                    :, c, (eo % 4) * P:(eo % 4 + 1) * P],
                                attn_T3_box[0][:, c, s0:s1],
                                start=(c == 0),
                                stop=(c == NCH - 1),
                            )
                    if split_out and q == 1:
                        nc.scalar.copy(osb[:, 0:256], ps[:, 0:256])
                        nc.sync.dma_start(
                            outT[ts(eo, P), lo:lo + 256], osb[:, 0:256])
                if o_bias:
                    for s0, s1, m in _segs(lo, lo + 512, split):
                        nc.scalar.activation(
                            osb[:, s0 - lo:s1 - lo], ps[:, s0 - lo:s1 - lo],
                            AF.Identity,
                            bias=c2_sb[:, m * NCH + eo:m * NCH + eo + 1],
                        )
                    nc.sync.dma_start(outT[ts(eo, P), lo:lo + 512], osb[:])
                elif split_out:
                    nc.scalar.copy(osb[:, 256:512], ps[:, 256:512])
                    nc.sync.dma_start(
                        outT[ts(eo, P), lo + 256:lo + 512], osb[:, 256:512])
                else:
                    if half == 0:
                        nc.vector.tensor_copy(osb[:], ps[:])
                    else:
                        # epilogue: Act idles once the exp stream has drained
                        nc.scalar.copy(osb[:], ps[:])
                    nc.sync.dma_start(outT[ts(eo, P), lo:lo + 512], osb[:])

            def pv_interleaved(pair, half, prs, nxt, fillers=(),
                               with_stats=False, mask_on_pool=False):
                """PV j-blocks for (pair, half) interleaved with the group
                two slots ahead's scores and PE filler work."""
                nxt_prs = []
                fillers = list(fillers)
                for j in range(4):
                    pv_j(pair, half, j, prs, with_stats=with_stats)
                    if fillers:
                        fillers.pop(0)()
                    if nxt is not None:
                        np_, nh = nxt
                        nxt_prs.append(scores_c(np_, nh, 2 * j, mask_on_pool))
                        nxt_prs.append(
                            scores_c(np_, nh, 2 * j + 1, mask_on_pool))
                for f in fillers:
                    f()
                return nxt_prs

            # ------------------------- group schedule ---------------------
            # prologue: full projections for pairs 0-2, scores for the first
            # three groups (the pipeline runs three groups ahead so the Act
            # exp stream never starves), v projection for pairs 0-1.
            prss = {}
            emit_qk_half(0, 0, "q")
            emit_qk_half(0, 0, "k")
            emit_qk_half(0, 1, "k")
            prss[(0, 0)] = [scores_c(0, 0, c) for c in range(4)]
            emit_qk_half(0, 1, "q")
            emit_qk_half(1, 0, "q")
            emit_qk_half(1, 0, "k")
            prss[(0, 0)] += [scores_c(0, 0, c) for c in range(4, 8)]
            emit_qk_half(1, 1, "k")
            emit_qk_half(1, 1, "q")
            dma_xv()
            fetch_wv_pair(0)
            fetch_wv_pair(1)
            prss[(1, 0)] = scores_group(1, 0)
            fetch_wqk(2)
            emit_qk_half(2, 0, "q")
            emit_qk_half(2, 0, "k")
            emit_v_pair(0)
            emit_qk_half(2, 1, "k")
            emit_qk_half(2, 1, "q")
            fetch_wqk(3)
            fetch_wv_pair(2)
            prss[(2, 0)] = scores_group(2, 0)
            emit_v_pair(1)
            dma_em_half(512)

            def qhs(eo):
                def f():
                    if eo + 1 < NCH:
                        fetch_wqk(eo + 1)
                    for half, name in ((0, "q"), (0, "k"), (1, "k"), (1, "q")):
                        emit_qk_half(eo, half, name)
                return f

            def vp(pair):
                def f():
                    emit_v_pair(pair)
                    if pair + 1 < NCH:
                        fetch_wv_pair(pair + 1)
                return f

            def op(half, eo):
                return lambda: outproj_eo(half, eo)

            def slot(p, h, np_, nh, fillers=(), with_stats=False,
                     mask_on_pool=False):
                nxt = (np_, nh) if np_ is not None else None
                nxt_prs = pv_interleaved(
                    p, h, prss.pop((p, h)), nxt, fillers,
                    with_stats=with_stats, mask_on_pool=mask_on_pool)
                if nxt is not None:
                    prss[nxt] = nxt_prs

            # ---- phase A: half-0 groups; q/k/v projections as filler ----
            # qhs(p) (as first filler) projects all four halves of pair p
            # right before that pair's scores and prefetches the next pair's
            # weights; vp(p) runs two slots ahead of its P@V group.
            slot(0, 0, 3, 0, [qhs(3), vp(2)])
            slot(1, 0, 4, 0, [qhs(4), vp(3)])
            slot(2, 0, 5, 0, [qhs(5), vp(4)])
            slot(3, 0, 6, 0, [qhs(6), vp(5)])
            slot(4, 0, 7, 0, [qhs(7), vp(6)])
            slot(5, 0, 0, 1, [vp(7)])
            xvwv.close()
            x_stack.close()
            fetch_wg()
            slot(6, 0, 1, 1)
            slot(7, 0, 2, 1)

            def chain(jj):
                def f():
                    chunk_stats(jj)
                    apply_chunk(jj)
                    transpose_chunk(jj)
                return f

            # ---- phase B: half-1 groups; the half-0 LN chains (emitted
            # here so they queue behind, not ahead of, the early half-1
            # mask/evac work), half-0 out-projection as filler.
            slot(0, 1, 3, 1, [chain(0), chain(1)])
            slot(1, 1, 4, 1, [chain(2), chain(3), op(0, 0)])
            slot(2, 1, 5, 1, [op(0, 1)])
            slot(3, 1, 6, 1, [op(0, 2)])
            slot(4, 1, 7, 1, [op(0, 3), op(0, 4)])
            slot(5, 1, None, None, [op(0, 5)])
            slot(6, 1, None, None, [op(0, 6)])
            outproj_eo(0, 7)

            # ---- epilogue: last group with inline stats+apply+transpose;
            # the half-1 out-projections' quarter accumulation groups gate
            # on the individual transposes, so emitting them after the last
            # transpose still overlaps the chunk 4-6 chains.
            prs71 = prss.pop((7, 1))
            pv_j(7, 1, 0, prs71, with_stats=True)
            pv_j(7, 1, 1, prs71, with_stats=True)
            pv_j(7, 1, 2, prs71, with_stats=True)
            pv_j(7, 1, 3, prs71, with_stats=True)
            for eo in range(NCH):
                outproj_eo(1, eo)

    nc.compile()
    return nc


def _pack_pmajor(arr2d):
    # [NCH*P, T] -> [P, NCH*T]: row p holds chunk-major concatenation
    return np.ascontiguousarray(
        arr2d.reshape(NCH, P, T).transpose(1, 0, 2).reshape(P, NCH * T)
    )


def _pack_wv_style(Wt, Wi):
    # [2, eoh, p, c*512+j] with arr[c*128+p, eoh*512+j], arr = W.T
    out = np.empty((2, 2, P, NCH * 512), NPBF16)
    for m, W in enumerate((Wt, Wi)):
        arr = (W.T).astype(NPBF16)  # [e_in, e_out]
        out[m] = (
            arr.reshape(NCH, P, 2, 512)
            .transpose(2, 1, 0, 3)
            .reshape(2, P, NCH * 512)
        )
    return np.ascontiguousarray(out)


def _host_prep(inputs):
    scaling = HD ** -0.5
    f32 = np.float32

    def a(name):
        return np.asarray(inputs[name], f32)

    def prep_blocks(Wt, Wi, scale=1.0):
        # [2, eo, p, c*128+j] with arr[c*128+p, eo*128+j]
        out = np.empty((2, NCH, P, NCH * P), NPBF16)
        for m, W in enumerate((Wt, Wi)):
            arr = ((W * scale).T).astype(NPBF16)  # [e_in, e_out]
            out[m] = (
                arr.reshape(NCH, P, NCH, P)
                .transpose(2, 1, 0, 3)
                .reshape(NCH, P, NCH * P)
            )
        return np.ascontiguousarray(out)

    Wo_t, Wo_i = a("Wo_t"), a("Wo_i")
    g_t, g_i = a("ln_g_t"), a("ln_g_i")
    b_t, b_i = a("ln_b_t"), a("ln_b_i")
    Wg_t = Wo_t * g_t[None, :]
    Wg_i = Wo_i * g_i[None, :]

    wq_np = prep_blocks(a("Wq_t"), a("Wq_i"), scaling)
    wk_np = prep_blocks(a("Wk_t"), a("Wk_i"))
    wg_np = _pack_wv_style(Wg_t, Wg_i)
    wv_np = _pack_wv_style(a("Wv_t"), a("Wv_i"))

    em_np = _pack_pmajor(
        np.exp(np.asarray(inputs["attention_mask"], np.float64)).T.astype(NPBF16)
    )

    bq_np = np.stack([a("bq_t"), a("bq_i")]) * f32(scaling)
    bk_np = np.stack([a("bk_t"), a("bk_i")])
    bv_np = np.stack([a("bv_t"), a("bv_i")])
    c2_np = np.stack(
        [
            Wo_t.astype(np.float64) @ b_t.astype(np.float64) + a("bo_t"),
            Wo_i.astype(np.float64) @ b_i.astype(np.float64) + a("bo_i"),
        ]
    ).astype(f32)
    ones_np = np.ones((1, P), np.float32)

    shared = dict(
        wq=wq_np, wk=wk_np, wg=wg_np, wv=wv_np, em=em_np,
        ones_d=ones_np,
        bq=np.ascontiguousarray(bq_np), bk=np.ascontiguousarray(bk_np),
        bv=np.ascontiguousarray(bv_np), c2=np.ascontiguousarray(c2_np),
    )
    flags = (
        bool(np.any(bv_np)),
        bool(np.any(bq_np) or np.any(bk_np)),
        bool(np.any(c2_np)),
    )
    return shared, flags


_CACHE = {}


def build_cached(split, flags):
    key = (split, flags)
    if key not in _CACHE:
        _CACHE[key] = build_module(split, *flags)
    return _CACHE[key]


def kernel(**inputs):
    q = np.asarray(inputs["query"], np.float32)
    k = np.asarray(inputs["key"], np.float32)
    v = np.asarray(inputs["value"], np.float32)
    assert q.shape == (B, T, E), q.shape
    split = int(np.asarray(inputs["split_position"]))

    shared, flags = _host_prep(inputs)
    nc = build_cached(split, flags)

    in_maps = []
    for b in range(B):
        m = dict(shared)
        m["xqT"] = _pack_pmajor(q[b].T.astype(NPBF16))
        m["xkT"] = _pack_pmajor(k[b].T.astype(NPBF16))
        m["xvT"] = _pack_pmajor(v[b].T.astype(NPBF16))
        in_maps.append(m)

    res = run_bass_kernel_spmd(nc, in_maps, list(range(B)))
    out = np.stack(
        [np.ascontiguousarray(res.results[b]["outT"].T) for b in range(B)]
    )
    return out.astype(np.float32)
